# revision 3
# baseline (speedup 1.0000x reference)
"""Multi-head attention on 8 Trainium2 cores.

Sharding: core c handles batch b = c // 4 and a quad of 4 heads
(hq = c % 4 -> heads 4*hq .. 4*hq+3) as two head-pairs of 64+64 = 128
partitions. w_q/w_k/w_v are split column-wise by head (tensor parallel),
w_out row-wise; per-batch partial outputs are summed on host.

Per-core pipeline (all matmuls fp32r, N>=256 so 1 cycle/row):
  A: qhT/khT [128, 2048] = w.T-stationary matmuls over 8 d_model chunks;
     vh natural [j, dh] via vT-stationary matmuls, with a ones column
     appended per head (attn @ [vh | 1] yields softmax denominators).
  B: scoresT [j 128, q 512] = khT-chunk-stationary matmul (K=64), then
     ACT exp(x * 0.125) PSUM->SBUF.
  C: outT_aug [65, 512] accumulated over 16 j-chunks; row 64 = denom.
  Normalize: DVE reciprocal + gpsimd partition broadcast + DVE multiply.
  D: out [q 128, 1024] = outnT-stationary matmul over both head pairs
     (K=256), PSUM->SBUF->DMA.
"""

import numpy as np

B = 2
S = 2048
D = 1024
NH = 16
DH = 64
HEADS_PER_CORE = 4
N_CORES = 8

_NC = None


def _build():
    import concourse.bacc as bacc
    import concourse.tile as tile
    import concourse.mybir as mybir

    fp32 = mybir.dt.float32
    fp32r = mybir.dt.float32r
    add = mybir.AluOpType.add
    mult = mybir.AluOpType.mult
    Exp = mybir.ActivationFunctionType.Exp

    nc = bacc.Bacc("TRN2", target_bir_lowering=False)

    qT = nc.dram_tensor("qT", (D, S), fp32, kind="ExternalInput")
    kT = nc.dram_tensor("kT", (D, S), fp32, kind="ExternalInput")
    vT = nc.dram_tensor("vT", (D, S), fp32, kind="ExternalInput")
    wq = nc.dram_tensor("wq", (D, 256), fp32, kind="ExternalInput")
    wk = nc.dram_tensor("wk", (D, 256), fp32, kind="ExternalInput")
    wv = nc.dram_tensor("wv", (D, 256), fp32, kind="ExternalInput")
    wo = nc.dram_tensor("wo", (256, D), fp32, kind="ExternalInput")
    bq = nc.dram_tensor("bq", (256, 1), fp32, kind="ExternalInput")
    bk = nc.dram_tensor("bk", (256, 1), fp32, kind="ExternalInput")
    bv = nc.dram_tensor("bv", (1, 256), fp32, kind="ExternalInput")
    out = nc.dram_tensor("out", (S, D), fp32, kind="ExternalOutput")

    def r(ap):
        return ap.bitcast(fp32r)

    with tile.TileContext(nc) as tc:
        with tc.tile_pool(name="persist", bufs=1) as P:
            qhT = [P.tile((128, S), fp32r, name=f"qhT{p}") for p in range(2)]
            khT = [P.tile((128, S), fp32r, name=f"khT{p}") for p in range(2)]
            vh = [P.tile((128, 16 * 65), fp32r, name=f"vh{h}") for h in range(4)]
            outnT = [P.tile((128, S), fp32r, name=f"outnT{p}") for p in range(2)]
            wq_sb = P.tile((128, 8 * 256), fp32r, name="wq_sb")
            wk_sb = P.tile((128, 8 * 256), fp32r, name="wk_sb")
            wv_sb = P.tile((128, 8 * 256), fp32r, name="wv_sb")
            wo_sb = [P.tile((128, D), fp32r, name=f"wo_sb{p}") for p in range(2)]
            bq_sb = P.tile((128, 2), fp32, name="bq_sb")
            bk_sb = P.tile((128, 2), fp32, name="bk_sb")
            bv_row = P.tile((1, 256), fp32, name="bv_row")
            bv_bc = P.tile((128, 256), fp32, name="bv_bc")
            ones_f = P.tile((128, 16 * 65), fp32, name="ones_f")

            nc.gpsimd.memset(ones_f[:], 1.0)
            for h in range(4):
                nc.vector.tensor_scalar(
                    vh[h][:], ones_f[:], 1.0, None, op0=mybir.AluOpType.mult
                )

            for kc in range(8):
                nc.sync.dma_start(
                    wq_sb[:, kc * 256:(kc + 1) * 256], r(wq[kc * 128:(kc + 1) * 128, :])
                )
                nc.sync.dma_start(
                    wk_sb[:, kc * 256:(kc + 1) * 256], r(wk[kc * 128:(kc + 1) * 128, :])
                )
                nc.sync.dma_start(
                    wv_sb[:, kc * 256:(kc + 1) * 256], r(wv[kc * 128:(kc + 1) * 128, :])
                )
            for p in range(2):
                nc.sync.dma_start(wo_sb[p][:], r(wo[p * 128:(p + 1) * 128, :]))
                nc.sync.dma_start(bq_sb[:, p:p + 1], bq[p * 128:(p + 1) * 128, :])
                nc.sync.dma_start(bk_sb[:, p:p + 1], bk[p * 128:(p + 1) * 128, :])
            nc.sync.dma_start(bv_row[:], bv[:])
            nc.gpsimd.partition_broadcast(bv_bc[:], bv_row[:])

            # ---- Stage A ----
            with tc.tile_pool(name="xin", bufs=2) as XP, \
                 tc.tile_pool(name="psA", bufs=2, space="PSUM") as PA, \
                 tc.tile_pool(name="psV", bufs=2, space="PSUM") as PV:
                for xdram, w_sb, b_sb, dstT in (
                    (qT, wq_sb, bq_sb, qhT),
                    (kT, wk_sb, bk_sb, khT),
                ):
                    for ns in range(4):
                        xt = XP.tile((128, 8 * 512), fp32r, name="xt")
                        for kc in range(8):
                            nc.sync.dma_start(
                                xt[:, kc * 512:(kc + 1) * 512],
                                r(xdram[kc * 128:(kc + 1) * 128, ns * 512:(ns + 1) * 512]),
                            )
                        for p in range(2):
                            ps = PA.tile((128, 512), fp32, name="psa")
                            for kc in range(8):
                                nc.tensor.matmul(
                                    ps[:],
                                    w_sb[:, kc * 256 + p * 128:kc * 256 + (p + 1) * 128],
                                    xt[:, kc * 512:(kc + 1) * 512],
                                    start=(kc == 0),
                                    stop=(kc == 7),
                                )
                            nc.vector.tensor_scalar_add(
                                dstT[p][:, ns * 512:(ns + 1) * 512], ps[:], b_sb[:, p:p + 1]
                            )
                # vh natural layout
                for ns in range(4):
                    xt = XP.tile((128, 8 * 512), fp32r, name="xt")
                    for kc in range(8):
                        nc.sync.dma_start(
                            xt[:, kc * 512:(kc + 1) * 512],
                            r(vT[kc * 128:(kc + 1) * 128, ns * 512:(ns + 1) * 512]),
                        )
                    for jj in range(4):
                        jc = ns * 4 + jj
                        ps = PV.tile((128, 256), fp32, name="psv")
                        for kc in range(8):
                            nc.tensor.matmul(
                                ps[:],
                                xt[:, kc * 512 + jj * 128:kc * 512 + (jj + 1) * 128],
                                wv_sb[:, kc * 256:(kc + 1) * 256],
                                start=(kc == 0),
                                stop=(kc == 7),
                            )
                        for h in range(4):
                            nc.vector.scalar_tensor_tensor(
                                vh[h][:, jc * 65:jc * 65 + 64],
                                ps[:, h * 64:(h + 1) * 64],
                                1.0,
                                bv_bc[:, h * 64:(h + 1) * 64],
                                op0=mult,
                                op1=add,
                            )

            # ---- Main loop: B (scores+exp), C (attn@V), D (out proj) ----
            with tc.tile_pool(name="psS", bufs=3, space="PSUM") as PS, \
                 tc.tile_pool(name="psC", bufs=2, space="PSUM") as PC, \
                 tc.tile_pool(name="psD", bufs=2, space="PSUM") as PD, \
                 tc.tile_pool(name="expP", bufs=3) as EP, \
                 tc.tile_pool(name="nrm", bufs=2) as NP, \
                 tc.tile_pool(name="outP", bufs=2) as OP:

                def emit_d(qs):
                    for qq in range(4):
                        qc0 = qs * 512 + qq * 128
                        osb = OP.tile((128, D), fp32, name="osb")
                        for ns in range(2):
                            dps = PD.tile((128, 512), fp32, name="dps")
                            for p in range(2):
                                nc.tensor.matmul(
                                    dps[:],
                                    outnT[p][:, qc0:qc0 + 128],
                                    wo_sb[p][:, ns * 512:(ns + 1) * 512],
                                    start=(p == 0),
                                    stop=(p == 1),
                                )
                            nc.vector.tensor_scalar_add(
                                osb[:, ns * 512:(ns + 1) * 512], dps[:], 0.0
                            )
                        nc.sync.dma_start(out[qc0:qc0 + 128, :], osb[:])

                for qs in range(4):
                    for h in range(4):
                        p, off = h // 2, (h % 2) * 64
                        cps = PC.tile((65, 512), fp32, name="cps")
                        prev = None
                        for jc in range(16):
                            sps = PS.tile((128, 512), fp32, name="sps")
                            nc.tensor.matmul(
                                sps[:],
                                khT[p][off:off + 64, jc * 128:(jc + 1) * 128],
                                qhT[p][off:off + 64, qs * 512:(qs + 1) * 512],
                                start=True,
                                stop=True,
                            )
                            ex = EP.tile((128, 512), fp32r, name="ex")
                            nc.scalar.activation(ex[:], sps[:], Exp, bias=0.0, scale=0.125)
                            if prev is not None:
                                pjc, pex = prev
                                nc.tensor.matmul(
                                    cps[:], vh[h][:, pjc * 65:(pjc + 1) * 65], pex[:],
                                    start=(pjc == 0), stop=False,
                                )
                            prev = (jc, ex)
                        nc.tensor.matmul(
                            cps[:], vh[h][:, 15 * 65:16 * 65], prev[1][:],
                            start=False, stop=True,
                        )
                        rec = NP.tile((1, 512), fp32, name="rec")
                        nc.vector.reciprocal(rec[:], cps[64:65, :])
                        rbc = NP.tile((64, 512), fp32, name="rbc")
                        nc.gpsimd.partition_broadcast(rbc[:], rec[:])
                        nc.vector.scalar_tensor_tensor(
                            outnT[p][off:off + 64, qs * 512:(qs + 1) * 512],
                            cps[0:64, :],
                            1.0,
                            rbc[:],
                            op0=mult,
                            op1=mult,
                        )
                        if h == 0 and qs > 0:
                            emit_d(qs - 1)
                emit_d(3)

    nc.compile()
    return nc


def _get_nc():
    global _NC
    if _NC is None:
        _NC = _build()
    return _NC


def run(inputs, trace=False, trace_cores=None):
    from concourse.bass_utils import run_bass_kernel_spmd

    q = np.asarray(inputs["q"], np.float32)
    k = np.asarray(inputs["k"], np.float32)
    v = np.asarray(inputs["v"], np.float32)
    w_q = np.asarray(inputs["w_q"], np.float32)
    w_k = np.asarray(inputs["w_k"], np.float32)
    w_v = np.asarray(inputs["w_v"], np.float32)
    w_out = np.asarray(inputs["w_out"], np.float32)
    b_q = np.asarray(inputs["b_q"], np.float32)
    b_k = np.asarray(inputs["b_k"], np.float32)
    b_v = np.asarray(inputs["b_v"], np.float32)
    b_out = np.asarray(inputs["b_out"], np.float32)

    xT = {b: {} for b in range(B)}
    for b in range(B):
        xT[b]["qT"] = np.ascontiguousarray(q[b].T)
        xT[b]["kT"] = np.ascontiguousarray(k[b].T)
        xT[b]["vT"] = np.ascontiguousarray(v[b].T)

    in_maps = []
    for c in range(N_CORES):
        b, hq = c // 4, c % 4
        rows = slice(hq * 256, (hq + 1) * 256)
        in_maps.append({
            "qT": xT[b]["qT"],
            "kT": xT[b]["kT"],
            "vT": xT[b]["vT"],
            "wq": np.ascontiguousarray(w_q[rows, :].T),
            "wk": np.ascontiguousarray(w_k[rows, :].T),
            "wv": np.ascontiguousarray(w_v[rows, :].T),
            "wo": np.ascontiguousarray(w_out[:, rows].T),
            "bq": np.ascontiguousarray(b_q[rows].reshape(256, 1)),
            "bk": np.ascontiguousarray(b_k[rows].reshape(256, 1)),
            "bv": np.ascontiguousarray(b_v[rows].reshape(1, 256)),
        })

    nc = _get_nc()
    res = run_bass_kernel_spmd(
        nc, in_maps, core_ids=list(range(N_CORES)), trace=trace,
        trace_cores=trace_cores,
    )
    full = np.zeros((B, S, D), np.float32)
    for c in range(N_CORES):
        full[c // 4] += np.asarray(res.results[c]["out"])
    full += b_out.reshape(1, 1, D)
    return full, res.exec_time_ns


def kernel(**inputs):
    return run(inputs, trace=False)[0]


# revision 4
# speedup vs baseline: 1.1826x; 1.1826x over previous
"""Multi-head attention on 8 Trainium2 cores.

Sharding: core c handles batch b = c // 4 and a quad of 4 heads
(hq = c % 4 -> heads 4*hq .. 4*hq+3) as two head-pairs of 64+64 = 128
partitions. w_q/w_k/w_v are split column-wise by head (tensor parallel),
w_out row-wise; per-batch partial outputs are summed on host.

Per-core pipeline (all matmuls bf16 in / fp32 PSUM out, 1 cycle/row):
  A: qhT/khT [128, 2048] = w.T-stationary matmuls over 8 d_model chunks;
     vh natural [j, dh] via vT-stationary matmuls, with a ones column
     appended per head (attn @ [vh | 1] yields softmax denominators).
  B: scoresT [j 128, q 512] = khT-chunk-stationary matmul (K=64), then
     ACT exp(x * 0.125) PSUM->SBUF.
  C: outT_aug [65, 512] accumulated over 16 j-chunks; row 64 = denom.
  Normalize: DVE reciprocal + gpsimd partition broadcast + DVE multiply.
  D: out [q 128, 1024] = outnT-stationary matmul over both head pairs
     (K=256), PSUM->SBUF->DMA.
"""

import numpy as np

B = 2
S = 2048
D = 1024
NH = 16
DH = 64
HEADS_PER_CORE = 4
N_CORES = 8

_NC = None


def _build():
    import concourse.bacc as bacc
    import concourse.tile as tile
    import concourse.mybir as mybir

    fp32 = mybir.dt.float32
    bf16 = mybir.dt.bfloat16
    add = mybir.AluOpType.add
    mult = mybir.AluOpType.mult
    Exp = mybir.ActivationFunctionType.Exp

    nc = bacc.Bacc("TRN2", target_bir_lowering=False)

    qT = nc.dram_tensor("qT", (D, S), bf16, kind="ExternalInput")
    kT = nc.dram_tensor("kT", (D, S), bf16, kind="ExternalInput")
    vT = nc.dram_tensor("vT", (D, S), bf16, kind="ExternalInput")
    wq = nc.dram_tensor("wq", (D, 256), bf16, kind="ExternalInput")
    wk = nc.dram_tensor("wk", (D, 256), bf16, kind="ExternalInput")
    wv = nc.dram_tensor("wv", (D, 256), bf16, kind="ExternalInput")
    wo = nc.dram_tensor("wo", (256, D), bf16, kind="ExternalInput")
    bq = nc.dram_tensor("bq", (256, 1), fp32, kind="ExternalInput")
    bk = nc.dram_tensor("bk", (256, 1), fp32, kind="ExternalInput")
    bv = nc.dram_tensor("bv", (1, 256), fp32, kind="ExternalInput")
    out = nc.dram_tensor("out", (S, D), fp32, kind="ExternalOutput")

    with tile.TileContext(nc) as tc:
        with tc.tile_pool(name="persist", bufs=1) as P:
            qhT = [P.tile((128, S), bf16, name=f"qhT{p}") for p in range(2)]
            khT = [P.tile((128, S), bf16, name=f"khT{p}") for p in range(2)]
            vh = [P.tile((128, 16 * 65), bf16, name=f"vh{h}") for h in range(4)]
            outnT = [P.tile((128, S), bf16, name=f"outnT{p}") for p in range(2)]
            wq_sb = P.tile((128, 8 * 256), bf16, name="wq_sb")
            wk_sb = P.tile((128, 8 * 256), bf16, name="wk_sb")
            wv_sb = P.tile((128, 8 * 256), bf16, name="wv_sb")
            wo_sb = [P.tile((128, D), bf16, name=f"wo_sb{p}") for p in range(2)]
            bq_sb = P.tile((128, 2), fp32, name="bq_sb")
            bk_sb = P.tile((128, 2), fp32, name="bk_sb")
            bv_row = P.tile((1, 256), fp32, name="bv_row")
            bv_bc = P.tile((128, 256), fp32, name="bv_bc")
            ones_f = P.tile((128, 16 * 65), fp32, name="ones_f")

            nc.gpsimd.memset(ones_f[:], 1.0)
            for h in range(4):
                nc.vector.tensor_scalar(
                    vh[h][:], ones_f[:], 1.0, None, op0=mybir.AluOpType.mult
                )

            for kc in range(8):
                nc.sync.dma_start(
                    wq_sb[:, kc * 256:(kc + 1) * 256], wq[kc * 128:(kc + 1) * 128, :]
                )
                nc.sync.dma_start(
                    wk_sb[:, kc * 256:(kc + 1) * 256], wk[kc * 128:(kc + 1) * 128, :]
                )
                nc.sync.dma_start(
                    wv_sb[:, kc * 256:(kc + 1) * 256], wv[kc * 128:(kc + 1) * 128, :]
                )
            for p in range(2):
                nc.sync.dma_start(wo_sb[p][:], wo[p * 128:(p + 1) * 128, :])
                nc.sync.dma_start(bq_sb[:, p:p + 1], bq[p * 128:(p + 1) * 128, :])
                nc.sync.dma_start(bk_sb[:, p:p + 1], bk[p * 128:(p + 1) * 128, :])
            nc.sync.dma_start(bv_row[:], bv[:])
            nc.gpsimd.partition_broadcast(bv_bc[:], bv_row[:])

            # ---- Stage A ----
            with tc.tile_pool(name="xin", bufs=2) as XP, \
                 tc.tile_pool(name="psA", bufs=2, space="PSUM") as PA, \
                 tc.tile_pool(name="psV", bufs=2, space="PSUM") as PV:
                for xdram, w_sb, b_sb, dstT in (
                    (qT, wq_sb, bq_sb, qhT),
                    (kT, wk_sb, bk_sb, khT),
                ):
                    for ns in range(4):
                        xt = XP.tile((128, 8 * 512), bf16, name="xt")
                        for kc in range(8):
                            nc.sync.dma_start(
                                xt[:, kc * 512:(kc + 1) * 512],
                                xdram[kc * 128:(kc + 1) * 128, ns * 512:(ns + 1) * 512],
                            )
                        for p in range(2):
                            ps = PA.tile((128, 512), fp32, name="psa")
                            for kc in range(8):
                                nc.tensor.matmul(
                                    ps[:],
                                    w_sb[:, kc * 256 + p * 128:kc * 256 + (p + 1) * 128],
                                    xt[:, kc * 512:(kc + 1) * 512],
                                    start=(kc == 0),
                                    stop=(kc == 7),
                                )
                            nc.vector.tensor_scalar_add(
                                dstT[p][:, ns * 512:(ns + 1) * 512], ps[:], b_sb[:, p:p + 1]
                            )
                # vh natural layout
                for ns in range(4):
                    xt = XP.tile((128, 8 * 512), bf16, name="xt")
                    for kc in range(8):
                        nc.sync.dma_start(
                            xt[:, kc * 512:(kc + 1) * 512],
                            vT[kc * 128:(kc + 1) * 128, ns * 512:(ns + 1) * 512],
                        )
                    for jj in range(4):
                        jc = ns * 4 + jj
                        ps = PV.tile((128, 256), fp32, name="psv")
                        for kc in range(8):
                            nc.tensor.matmul(
                                ps[:],
                                xt[:, kc * 512 + jj * 128:kc * 512 + (jj + 1) * 128],
                                wv_sb[:, kc * 256:(kc + 1) * 256],
                                start=(kc == 0),
                                stop=(kc == 7),
                            )
                        for h in range(4):
                            nc.vector.scalar_tensor_tensor(
                                vh[h][:, jc * 65:jc * 65 + 64],
                                ps[:, h * 64:(h + 1) * 64],
                                1.0,
                                bv_bc[:, h * 64:(h + 1) * 64],
                                op0=mult,
                                op1=add,
                            )

            # ---- Main loop: B (scores+exp), C (attn@V), D (out proj) ----
            with tc.tile_pool(name="psS", bufs=3, space="PSUM") as PS, \
                 tc.tile_pool(name="psC", bufs=2, space="PSUM") as PC, \
                 tc.tile_pool(name="psD", bufs=2, space="PSUM") as PD, \
                 tc.tile_pool(name="expP", bufs=3) as EP, \
                 tc.tile_pool(name="nrm", bufs=2) as NP, \
                 tc.tile_pool(name="outP", bufs=2) as OP:

                def emit_d(qs):
                    for qq in range(4):
                        qc0 = qs * 512 + qq * 128
                        osb = OP.tile((128, D), fp32, name="osb")
                        for ns in range(2):
                            dps = PD.tile((128, 512), fp32, name="dps")
                            for p in range(2):
                                nc.tensor.matmul(
                                    dps[:],
                                    outnT[p][:, qc0:qc0 + 128],
                                    wo_sb[p][:, ns * 512:(ns + 1) * 512],
                                    start=(p == 0),
                                    stop=(p == 1),
                                )
                            nc.vector.tensor_scalar_add(
                                osb[:, ns * 512:(ns + 1) * 512], dps[:], 0.0
                            )
                        nc.sync.dma_start(out[qc0:qc0 + 128, :], osb[:])

                for qs in range(4):
                    for h in range(4):
                        p, off = h // 2, (h % 2) * 64
                        cps = PC.tile((65, 512), fp32, name="cps")
                        prev = None
                        for jc in range(16):
                            sps = PS.tile((128, 512), fp32, name="sps")
                            nc.tensor.matmul(
                                sps[:],
                                khT[p][off:off + 64, jc * 128:(jc + 1) * 128],
                                qhT[p][off:off + 64, qs * 512:(qs + 1) * 512],
                                start=True,
                                stop=True,
                            )
                            ex = EP.tile((128, 512), bf16, name="ex")
                            nc.scalar.activation(ex[:], sps[:], Exp, bias=0.0, scale=0.125)
                            if prev is not None:
                                pjc, pex = prev
                                nc.tensor.matmul(
                                    cps[:], vh[h][:, pjc * 65:(pjc + 1) * 65], pex[:],
                                    start=(pjc == 0), stop=False,
                                )
                            prev = (jc, ex)
                        nc.tensor.matmul(
                            cps[:], vh[h][:, 15 * 65:16 * 65], prev[1][:],
                            start=False, stop=True,
                        )
                        rec = NP.tile((1, 512), fp32, name="rec")
                        nc.vector.reciprocal(rec[:], cps[64:65, :])
                        rbc = NP.tile((64, 512), fp32, name="rbc")
                        nc.gpsimd.partition_broadcast(rbc[:], rec[:])
                        nc.vector.scalar_tensor_tensor(
                            outnT[p][off:off + 64, qs * 512:(qs + 1) * 512],
                            cps[0:64, :],
                            1.0,
                            rbc[:],
                            op0=mult,
                            op1=mult,
                        )
                        if h == 0 and qs > 0:
                            emit_d(qs - 1)
                emit_d(3)

    nc.compile()
    return nc


def _get_nc():
    global _NC
    if _NC is None:
        _NC = _build()
    return _NC


def run(inputs, trace=False, trace_cores=None):
    from concourse.bass_utils import run_bass_kernel_spmd

    q = np.asarray(inputs["q"], np.float32)
    k = np.asarray(inputs["k"], np.float32)
    v = np.asarray(inputs["v"], np.float32)
    w_q = np.asarray(inputs["w_q"], np.float32)
    w_k = np.asarray(inputs["w_k"], np.float32)
    w_v = np.asarray(inputs["w_v"], np.float32)
    w_out = np.asarray(inputs["w_out"], np.float32)
    b_q = np.asarray(inputs["b_q"], np.float32)
    b_k = np.asarray(inputs["b_k"], np.float32)
    b_v = np.asarray(inputs["b_v"], np.float32)
    b_out = np.asarray(inputs["b_out"], np.float32)

    import ml_dtypes
    bf16 = ml_dtypes.bfloat16

    xT = {b: {} for b in range(B)}
    for b in range(B):
        xT[b]["qT"] = np.ascontiguousarray(q[b].T.astype(bf16))
        xT[b]["kT"] = np.ascontiguousarray(k[b].T.astype(bf16))
        xT[b]["vT"] = np.ascontiguousarray(v[b].T.astype(bf16))

    in_maps = []
    for c in range(N_CORES):
        b, hq = c // 4, c % 4
        rows = slice(hq * 256, (hq + 1) * 256)
        in_maps.append({
            "qT": xT[b]["qT"],
            "kT": xT[b]["kT"],
            "vT": xT[b]["vT"],
            "wq": np.ascontiguousarray(w_q[rows, :].T.astype(bf16)),
            "wk": np.ascontiguousarray(w_k[rows, :].T.astype(bf16)),
            "wv": np.ascontiguousarray(w_v[rows, :].T.astype(bf16)),
            "wo": np.ascontiguousarray(w_out[:, rows].T.astype(bf16)),
            "bq": np.ascontiguousarray(b_q[rows].reshape(256, 1)),
            "bk": np.ascontiguousarray(b_k[rows].reshape(256, 1)),
            "bv": np.ascontiguousarray(b_v[rows].reshape(1, 256)),
        })

    nc = _get_nc()
    res = run_bass_kernel_spmd(
        nc, in_maps, core_ids=list(range(N_CORES)), trace=trace,
        trace_cores=trace_cores,
    )
    full = np.zeros((B, S, D), np.float32)
    for c in range(N_CORES):
        full[c // 4] += np.asarray(res.results[c]["out"])
    full += b_out.reshape(1, 1, D)
    return full, res.exec_time_ns


def kernel(**inputs):
    return run(inputs, trace=False)[0]


# revision 6
# speedup vs baseline: 1.2985x; 1.0980x over previous
"""Multi-head attention on 8 Trainium2 cores.

Sharding: core c handles batch b = c // 4 and a quad of 4 heads
(hq = c % 4 -> heads 4*hq .. 4*hq+3) as two head-pairs of 64+64 = 128
partitions. w_q/w_k/w_v are split column-wise by head (tensor parallel),
w_out row-wise; per-batch partial outputs are summed on host.

Per-core pipeline (all matmuls bf16 in / fp32 PSUM out, 1 cycle/row):
  A: qhT/khT [128, 2048] = w.T-stationary matmuls over 8 d_model chunks;
     vh natural [j, dh] via vT-stationary matmuls, with a ones column
     appended per head (attn @ [vh | 1] yields softmax denominators).
  B: scoresT [j 128, q 512] = khT-chunk-stationary matmul (K=64), then
     ACT exp(x * 0.125) PSUM->SBUF.
  C: outT_aug [65, 512] accumulated over 16 j-chunks; row 64 = denom.
  Normalize: DVE reciprocal + gpsimd partition broadcast + DVE multiply.
  D: out [q 128, 1024] = outnT-stationary matmul over both head pairs
     (K=256), PSUM->SBUF->DMA.
"""

import numpy as np

B = 2
S = 2048
D = 1024
NH = 16
DH = 64
HEADS_PER_CORE = 4
N_CORES = 8

_NC = None


def _build():
    import concourse.bacc as bacc
    import concourse.tile as tile
    import concourse.mybir as mybir

    fp32 = mybir.dt.float32
    bf16 = mybir.dt.bfloat16
    add = mybir.AluOpType.add
    mult = mybir.AluOpType.mult
    Exp = mybir.ActivationFunctionType.Exp

    nc = bacc.Bacc("TRN2", target_bir_lowering=False)

    qT = nc.dram_tensor("qT", (D, S), bf16, kind="ExternalInput")
    kT = nc.dram_tensor("kT", (D, S), bf16, kind="ExternalInput")
    vT = nc.dram_tensor("vT", (D, S), bf16, kind="ExternalInput")
    wq = nc.dram_tensor("wq", (D, 256), bf16, kind="ExternalInput")
    wk = nc.dram_tensor("wk", (D, 256), bf16, kind="ExternalInput")
    wv = nc.dram_tensor("wv", (D, 256), bf16, kind="ExternalInput")
    wo = nc.dram_tensor("wo", (256, D), bf16, kind="ExternalInput")
    bq = nc.dram_tensor("bq", (256, 1), fp32, kind="ExternalInput")
    bk = nc.dram_tensor("bk", (256, 1), fp32, kind="ExternalInput")
    bv = nc.dram_tensor("bv", (1, 256), fp32, kind="ExternalInput")
    out = nc.dram_tensor("out", (S, D), fp32, kind="ExternalOutput")

    with tile.TileContext(nc) as tc:
        with tc.tile_pool(name="persist", bufs=1) as P:
            qhT = [P.tile((128, S), bf16, name=f"qhT{p}") for p in range(2)]
            khT = [P.tile((128, S), bf16, name=f"khT{p}") for p in range(2)]
            vh = [P.tile((128, 16 * 65), bf16, name=f"vh{h}") for h in range(4)]
            outnT = [P.tile((128, S), bf16, name=f"outnT{p}") for p in range(2)]
            wq_sb = P.tile((128, 8 * 256), bf16, name="wq_sb")
            wk_sb = P.tile((128, 8 * 256), bf16, name="wk_sb")
            wv_sb = P.tile((128, 8 * 256), bf16, name="wv_sb")
            wo_sb = [P.tile((128, D), bf16, name=f"wo_sb{p}") for p in range(2)]
            bq_sb = P.tile((128, 2), fp32, name="bq_sb")
            bk_sb = P.tile((128, 2), fp32, name="bk_sb")
            bv_row = P.tile((1, 256), fp32, name="bv_row")
            bv_bc = P.tile((128, 256), fp32, name="bv_bc")
            ones_f = P.tile((128, 16 * 65), fp32, name="ones_f")

            nc.gpsimd.memset(ones_f[:], 1.0)
            for h in range(4):
                nc.vector.tensor_scalar(
                    vh[h][:], ones_f[:], 1.0, None, op0=mybir.AluOpType.mult
                )

            for kc in range(8):
                nc.sync.dma_start(
                    wq_sb[:, kc * 256:(kc + 1) * 256], wq[kc * 128:(kc + 1) * 128, :]
                )
                nc.sync.dma_start(
                    wk_sb[:, kc * 256:(kc + 1) * 256], wk[kc * 128:(kc + 1) * 128, :]
                )
                nc.sync.dma_start(
                    wv_sb[:, kc * 256:(kc + 1) * 256], wv[kc * 128:(kc + 1) * 128, :]
                )
            for p in range(2):
                nc.sync.dma_start(wo_sb[p][:], wo[p * 128:(p + 1) * 128, :])
                nc.sync.dma_start(bq_sb[:, p:p + 1], bq[p * 128:(p + 1) * 128, :])
                nc.sync.dma_start(bk_sb[:, p:p + 1], bk[p * 128:(p + 1) * 128, :])
            nc.sync.dma_start(bv_row[:], bv[:])
            nc.gpsimd.partition_broadcast(bv_bc[:], bv_row[:])

            # ---- Stage A ----
            with tc.tile_pool(name="xin", bufs=2) as XP, \
                 tc.tile_pool(name="psA", bufs=2, space="PSUM") as PA, \
                 tc.tile_pool(name="psV", bufs=2, space="PSUM") as PV:
                for xdram, w_sb, b_sb, dstT in (
                    (qT, wq_sb, bq_sb, qhT),
                    (kT, wk_sb, bk_sb, khT),
                ):
                    for ns in range(4):
                        xt = XP.tile((128, 8 * 512), bf16, name="xt")
                        for kc in range(8):
                            nc.sync.dma_start(
                                xt[:, kc * 512:(kc + 1) * 512],
                                xdram[kc * 128:(kc + 1) * 128, ns * 512:(ns + 1) * 512],
                            )
                        for p in range(2):
                            ps = PA.tile((128, 512), fp32, name="psa")
                            for kc in range(8):
                                nc.tensor.matmul(
                                    ps[:],
                                    w_sb[:, kc * 256 + p * 128:kc * 256 + (p + 1) * 128],
                                    xt[:, kc * 512:(kc + 1) * 512],
                                    start=(kc == 0),
                                    stop=(kc == 7),
                                )
                            nc.vector.tensor_scalar_add(
                                dstT[p][:, ns * 512:(ns + 1) * 512], ps[:], b_sb[:, p:p + 1]
                            )
                # vh natural layout
                for ns in range(4):
                    xt = XP.tile((128, 8 * 512), bf16, name="xt")
                    for kc in range(8):
                        nc.sync.dma_start(
                            xt[:, kc * 512:(kc + 1) * 512],
                            vT[kc * 128:(kc + 1) * 128, ns * 512:(ns + 1) * 512],
                        )
                    for jj in range(4):
                        jc = ns * 4 + jj
                        ps = PV.tile((128, 256), fp32, name="psv")
                        for kc in range(8):
                            nc.tensor.matmul(
                                ps[:],
                                xt[:, kc * 512 + jj * 128:kc * 512 + (jj + 1) * 128],
                                wv_sb[:, kc * 256:(kc + 1) * 256],
                                start=(kc == 0),
                                stop=(kc == 7),
                            )
                        for h in range(4):
                            nc.vector.scalar_tensor_tensor(
                                vh[h][:, jc * 65:jc * 65 + 64],
                                ps[:, h * 64:(h + 1) * 64],
                                1.0,
                                bv_bc[:, h * 64:(h + 1) * 64],
                                op0=mult,
                                op1=add,
                            )

            # ---- Main loop: B (scores+exp), C (attn@V), D (out proj) ----
            with tc.tile_pool(name="psS", bufs=2, space="PSUM") as PS, \
                 tc.tile_pool(name="psC", bufs=2, space="PSUM") as PC, \
                 tc.tile_pool(name="psD", bufs=2, space="PSUM") as PD, \
                 tc.tile_pool(name="expP", bufs=3) as EP, \
                 tc.tile_pool(name="nrm", bufs=2) as NP, \
                 tc.tile_pool(name="outP", bufs=2) as OP:

                def emit_d(qs):
                    for qq in range(4):
                        qc0 = qs * 512 + qq * 128
                        osb = OP.tile((128, D), fp32, name="osb")
                        for ns in range(2):
                            dps = PD.tile((128, 512), fp32, name="dps")
                            for p in range(2):
                                nc.tensor.matmul(
                                    dps[:],
                                    outnT[p][:, qc0:qc0 + 128],
                                    wo_sb[p][:, ns * 512:(ns + 1) * 512],
                                    start=(p == 0),
                                    stop=(p == 1),
                                )
                            nc.vector.tensor_scalar_add(
                                osb[:, ns * 512:(ns + 1) * 512], dps[:], 0.0
                            )
                        nc.sync.dma_start(out[qc0:qc0 + 128, :], osb[:])

                for qs in range(4):
                    for h in range(4):
                        p, off = h // 2, (h % 2) * 64
                        cps = PC.tile((65, 512), fp32, name="cps")

                        def emit_c(pjp, pex):
                            for half in range(2):
                                pjc = pjp * 2 + half
                                nc.tensor.matmul(
                                    cps[:],
                                    vh[h][:, pjc * 65:(pjc + 1) * 65],
                                    pex[:, half * 512:(half + 1) * 512],
                                    start=(pjc == 0),
                                    stop=(pjc == 15),
                                )

                        prev = None
                        for jp in range(8):
                            sps = PS.tile((128, 1024), fp32, name="sps")
                            for half in range(2):
                                jc = jp * 2 + half
                                nc.tensor.matmul(
                                    sps[:, half * 512:(half + 1) * 512],
                                    khT[p][off:off + 64, jc * 128:(jc + 1) * 128],
                                    qhT[p][off:off + 64, qs * 512:(qs + 1) * 512],
                                    start=True,
                                    stop=True,
                                )
                            ex = EP.tile((128, 1024), bf16, name="ex")
                            nc.scalar.activation(ex[:], sps[:], Exp, bias=0.0, scale=0.125)
                            if prev is not None:
                                emit_c(*prev)
                            prev = (jp, ex)
                        emit_c(*prev)
                        rec = NP.tile((1, 512), fp32, name="rec")
                        nc.vector.reciprocal(rec[:], cps[64:65, :])
                        rbc = NP.tile((64, 512), fp32, name="rbc")
                        nc.gpsimd.partition_broadcast(rbc[:], rec[:])
                        nc.vector.scalar_tensor_tensor(
                            outnT[p][off:off + 64, qs * 512:(qs + 1) * 512],
                            cps[0:64, :],
                            1.0,
                            rbc[:],
                            op0=mult,
                            op1=mult,
                        )
                        if h == 0 and qs > 0:
                            emit_d(qs - 1)
                emit_d(3)

    nc.compile()
    return nc


def _get_nc():
    global _NC
    if _NC is None:
        _NC = _build()
    return _NC


def run(inputs, trace=False, trace_cores=None):
    from concourse.bass_utils import run_bass_kernel_spmd

    q = np.asarray(inputs["q"], np.float32)
    k = np.asarray(inputs["k"], np.float32)
    v = np.asarray(inputs["v"], np.float32)
    w_q = np.asarray(inputs["w_q"], np.float32)
    w_k = np.asarray(inputs["w_k"], np.float32)
    w_v = np.asarray(inputs["w_v"], np.float32)
    w_out = np.asarray(inputs["w_out"], np.float32)
    b_q = np.asarray(inputs["b_q"], np.float32)
    b_k = np.asarray(inputs["b_k"], np.float32)
    b_v = np.asarray(inputs["b_v"], np.float32)
    b_out = np.asarray(inputs["b_out"], np.float32)

    import ml_dtypes
    bf16 = ml_dtypes.bfloat16

    xT = {b: {} for b in range(B)}
    for b in range(B):
        xT[b]["qT"] = np.ascontiguousarray(q[b].T.astype(bf16))
        xT[b]["kT"] = np.ascontiguousarray(k[b].T.astype(bf16))
        xT[b]["vT"] = np.ascontiguousarray(v[b].T.astype(bf16))

    in_maps = []
    for c in range(N_CORES):
        b, hq = c // 4, c % 4
        rows = slice(hq * 256, (hq + 1) * 256)
        in_maps.append({
            "qT": xT[b]["qT"],
            "kT": xT[b]["kT"],
            "vT": xT[b]["vT"],
            "wq": np.ascontiguousarray(w_q[rows, :].T.astype(bf16)),
            "wk": np.ascontiguousarray(w_k[rows, :].T.astype(bf16)),
            "wv": np.ascontiguousarray(w_v[rows, :].T.astype(bf16)),
            "wo": np.ascontiguousarray(w_out[:, rows].T.astype(bf16)),
            "bq": np.ascontiguousarray(b_q[rows].reshape(256, 1)),
            "bk": np.ascontiguousarray(b_k[rows].reshape(256, 1)),
            "bv": np.ascontiguousarray(b_v[rows].reshape(1, 256)),
        })

    nc = _get_nc()
    res = run_bass_kernel_spmd(
        nc, in_maps, core_ids=list(range(N_CORES)), trace=trace,
        trace_cores=trace_cores,
    )
    full = np.zeros((B, S, D), np.float32)
    for c in range(N_CORES):
        full[c // 4] += np.asarray(res.results[c]["out"])
    full += b_out.reshape(1, 1, D)
    return full, res.exec_time_ns


def kernel(**inputs):
    return run(inputs, trace=False)[0]


# revision 11
# speedup vs baseline: 1.4186x; 1.0925x over previous
"""Multi-head attention on 8 Trainium2 cores.

Sharding: core c handles batch b = c // 4 and a quad of 4 heads
(hq = c % 4 -> heads 4*hq .. 4*hq+3) as two head-pairs of 64+64 = 128
partitions. w_q/w_k/w_v are split column-wise by head (tensor parallel),
w_out row-wise; per-batch partial outputs are summed on host.

Per-core pipeline (all matmuls bf16 in / fp32 PSUM out, 1 cycle/row):
  A: qhT/khT [128, 2048] = w.T-stationary matmuls over 8 d_model chunks;
     vh natural [j, dh] via vT-stationary matmuls, with a ones column
     appended per head (attn @ [vh | 1] yields softmax denominators).
  B: scoresT [j 128, q 512] = khT-chunk-stationary matmul (K=64), then
     ACT exp(x * 0.125) PSUM->SBUF.
  C: outT_aug [65, 512] accumulated over 16 j-chunks; row 64 = denom.
  Normalize: DVE reciprocal + gpsimd partition broadcast + DVE multiply.
  D: out [q 128, 1024] = outnT-stationary matmul over both head pairs
     (K=256), PSUM->SBUF->DMA.
"""

import numpy as np

B = 2
S = 2048
D = 1024
NH = 16
DH = 64
HEADS_PER_CORE = 4
N_CORES = 8

_NC = None


def _build():
    import concourse.bacc as bacc
    import concourse.tile as tile
    import concourse.mybir as mybir

    fp32 = mybir.dt.float32
    bf16 = mybir.dt.bfloat16
    add = mybir.AluOpType.add
    mult = mybir.AluOpType.mult
    Exp = mybir.ActivationFunctionType.Exp

    nc = bacc.Bacc("TRN2", target_bir_lowering=False)

    qT = nc.dram_tensor("qT", (D, S), bf16, kind="ExternalInput")
    kT = nc.dram_tensor("kT", (D, S), bf16, kind="ExternalInput")
    vT = nc.dram_tensor("vT", (D, S), bf16, kind="ExternalInput")
    wq = nc.dram_tensor("wq", (D, 256), bf16, kind="ExternalInput")
    wk = nc.dram_tensor("wk", (D, 256), bf16, kind="ExternalInput")
    wv = nc.dram_tensor("wv", (D, 256), bf16, kind="ExternalInput")
    wo = nc.dram_tensor("wo", (256, D), bf16, kind="ExternalInput")
    bq = nc.dram_tensor("bq", (256, 1), fp32, kind="ExternalInput")
    bk = nc.dram_tensor("bk", (256, 1), fp32, kind="ExternalInput")
    bv = nc.dram_tensor("bv", (1, 256), fp32, kind="ExternalInput")
    out = nc.dram_tensor("out", (S, D), fp32, kind="ExternalOutput")

    with tile.TileContext(nc) as tc:
        with tc.tile_pool(name="persist", bufs=1) as P:
            qhT = [P.tile((128, S), bf16, name=f"qhT{p}") for p in range(2)]
            khT = [P.tile((128, S), bf16, name=f"khT{p}") for p in range(2)]
            vh = [P.tile((128, 16 * 65), bf16, name=f"vh{h}") for h in range(4)]
            outnT = [P.tile((128, S), bf16, name=f"outnT{p}") for p in range(2)]
            wq_sb = P.tile((128, 8 * 256), bf16, name="wq_sb")
            wk_sb = P.tile((128, 8 * 256), bf16, name="wk_sb")
            wv_sb = P.tile((128, 8 * 256), bf16, name="wv_sb")
            wo_sb = [P.tile((128, D), bf16, name=f"wo_sb{p}") for p in range(2)]
            bq_sb = P.tile((128, 2), fp32, name="bq_sb")
            bk_sb = P.tile((128, 2), fp32, name="bk_sb")
            bv_row = P.tile((1, 256), fp32, name="bv_row")
            bv_bc = P.tile((128, 256), fp32, name="bv_bc")
            ones_f = P.tile((128, 16 * 65), fp32, name="ones_f")

            nc.gpsimd.memset(ones_f[:], 1.0)
            for h in range(4):
                nc.vector.tensor_scalar(
                    vh[h][:], ones_f[:], 1.0, None, op0=mybir.AluOpType.mult
                )

            for kc in range(8):
                nc.sync.dma_start(
                    wq_sb[:, kc * 256:(kc + 1) * 256], wq[kc * 128:(kc + 1) * 128, :]
                )
                nc.sync.dma_start(
                    wk_sb[:, kc * 256:(kc + 1) * 256], wk[kc * 128:(kc + 1) * 128, :]
                )
                nc.sync.dma_start(
                    wv_sb[:, kc * 256:(kc + 1) * 256], wv[kc * 128:(kc + 1) * 128, :]
                )
            for p in range(2):
                nc.sync.dma_start(wo_sb[p][:], wo[p * 128:(p + 1) * 128, :])
                nc.sync.dma_start(bq_sb[:, p:p + 1], bq[p * 128:(p + 1) * 128, :])
                nc.sync.dma_start(bk_sb[:, p:p + 1], bk[p * 128:(p + 1) * 128, :])
            nc.sync.dma_start(bv_row[:], bv[:])
            nc.gpsimd.partition_broadcast(bv_bc[:], bv_row[:])

            with tc.tile_pool(name="xin", bufs=2) as XP:

                def load_x(xdram, ns):
                    xt = XP.tile((128, 8 * 512), bf16, name="xt")
                    for kc in range(8):
                        nc.sync.dma_start(
                            xt[:, kc * 512:(kc + 1) * 512],
                            xdram[kc * 128:(kc + 1) * 128, ns * 512:(ns + 1) * 512],
                        )
                    return xt

                # ---- Upfront: khT (all), vh (all), qhT (qs=0 only) ----
                with tc.tile_pool(name="psA", bufs=2, space="PSUM") as PA, \
                     tc.tile_pool(name="psV", bufs=2, space="PSUM") as PV:

                    def proj_qk(xdram, w_sb, b_sb, dstT, ns):
                        xt = load_x(xdram, ns)
                        for p in range(2):
                            ps = PA.tile((128, 512), fp32, name="psa")
                            for kc in range(8):
                                nc.tensor.matmul(
                                    ps[:],
                                    w_sb[:, kc * 256 + p * 128:kc * 256 + (p + 1) * 128],
                                    xt[:, kc * 512:(kc + 1) * 512],
                                    start=(kc == 0),
                                    stop=(kc == 7),
                                )
                            nc.vector.tensor_scalar_add(
                                dstT[p][:, ns * 512:(ns + 1) * 512], ps[:], b_sb[:, p:p + 1]
                            )

                    for ns in range(4):
                        proj_qk(kT, wk_sb, bk_sb, khT, ns)
                    for ns in range(4):
                        xt = load_x(vT, ns)
                        for jj in range(4):
                            jc = ns * 4 + jj
                            ps = PV.tile((128, 256), fp32, name="psv")
                            for kc in range(8):
                                nc.tensor.matmul(
                                    ps[:],
                                    xt[:, kc * 512 + jj * 128:kc * 512 + (jj + 1) * 128],
                                    wv_sb[:, kc * 256:(kc + 1) * 256],
                                    start=(kc == 0),
                                    stop=(kc == 7),
                                )
                            for h in range(4):
                                nc.vector.scalar_tensor_tensor(
                                    vh[h][:, jc * 65:jc * 65 + 64],
                                    ps[:, h * 64:(h + 1) * 64],
                                    1.0,
                                    bv_bc[:, h * 64:(h + 1) * 64],
                                    op0=mult,
                                    op1=add,
                                )
                    proj_qk(qT, wq_sb, bq_sb, qhT, 0)

                # ---- Main loop: B+exp with interleaved filler (qh proj / D) ----
                with tc.tile_pool(name="psS", bufs=2, space="PSUM") as PS, \
                     tc.tile_pool(name="psC", bufs=2, space="PSUM") as PC, \
                     tc.tile_pool(name="psD", bufs=1, space="PSUM") as PD, \
                     tc.tile_pool(name="psF", bufs=1, space="PSUM") as PF, \
                     tc.tile_pool(name="expP", bufs=3) as EP, \
                     tc.tile_pool(name="nrm", bufs=2) as NP, \
                     tc.tile_pool(name="outP", bufs=2) as OP:

                    def qh_filler(ns):
                        xt = load_x(qT, ns)
                        for p in range(2):
                            ps = PF.tile((128, 512), fp32, name="psf")
                            for kc in range(8):
                                nc.tensor.matmul(
                                    ps[:],
                                    wq_sb[:, kc * 256 + p * 128:kc * 256 + (p + 1) * 128],
                                    xt[:, kc * 512:(kc + 1) * 512],
                                    start=(kc == 0),
                                    stop=(kc == 7),
                                )
                                yield
                            nc.vector.tensor_scalar_add(
                                qhT[p][:, ns * 512:(ns + 1) * 512], ps[:], bq_sb[:, p:p + 1]
                            )

                    def d_filler(qs):
                        for qq in range(4):
                            qc0 = qs * 512 + qq * 128
                            osb = OP.tile((128, D), fp32, name="osb")
                            for nsd in range(2):
                                dps = PD.tile((128, 512), fp32, name="dps")
                                for p in range(2):
                                    nc.tensor.matmul(
                                        dps[:],
                                        outnT[p][:, qc0:qc0 + 128],
                                        wo_sb[p][:, nsd * 512:(nsd + 1) * 512],
                                        start=(p == 0),
                                        stop=(p == 1),
                                    )
                                    yield
                                nc.vector.tensor_scalar_add(
                                    osb[:, nsd * 512:(nsd + 1) * 512], dps[:], 0.0
                                )
                            nc.sync.dma_start(out[qc0:qc0 + 128, :], osb[:])

                    def chain(*gens):
                        for g in gens:
                            yield from g

                    def emit_c(ph, pjp, pcps, pex):
                        for half in range(2):
                            pjc = pjp * 2 + half
                            nc.tensor.matmul(
                                pcps[:],
                                vh[ph][:, pjc * 65:(pjc + 1) * 65],
                                pex[:, half * 512:(half + 1) * 512],
                                start=(pjc == 0),
                                stop=(pjc == 15),
                            )

                    def normalize(ph, pqs, pcps):
                        p, off = ph // 2, (ph % 2) * 64
                        rec = NP.tile((1, 512), fp32, name="rec")
                        nc.vector.reciprocal(rec[:], pcps[64:65, :])
                        rbc = NP.tile((64, 512), fp32, name="rbc")
                        nc.gpsimd.partition_broadcast(rbc[:], rec[:])
                        nc.vector.scalar_tensor_tensor(
                            outnT[p][off:off + 64, pqs * 512:(pqs + 1) * 512],
                            pcps[0:64, :],
                            1.0,
                            rbc[:],
                            op0=mult,
                            op1=mult,
                        )

                    pending = [None]
                    for qs in range(4):
                        if qs == 0:
                            filler = chain(qh_filler(1), qh_filler(2))
                        elif qs == 1:
                            filler = chain(qh_filler(3), d_filler(0))
                        elif qs == 2:
                            filler = d_filler(1)
                        else:
                            filler = d_filler(2)
                        for h in range(4):
                            p, off = h // 2, (h % 2) * 64
                            cps = PC.tile((65, 512), fp32, name="cps")
                            for jp in range(8):
                                sps = PS.tile((128, 1024), fp32, name="sps")
                                for half in range(2):
                                    jc = jp * 2 + half
                                    nc.tensor.matmul(
                                        sps[:, half * 512:(half + 1) * 512],
                                        khT[p][off:off + 64, jc * 128:(jc + 1) * 128],
                                        qhT[p][off:off + 64, qs * 512:(qs + 1) * 512],
                                        start=True,
                                        stop=True,
                                    )
                                ex = EP.tile((128, 1024), bf16, name="ex")
                                nc.scalar.activation(ex[:], sps[:], Exp, bias=0.0, scale=0.125)

                                def consume():
                                    if pending[0] is not None:
                                        ph, pqs, pjp, pcps, pex = pending[0]
                                        emit_c(ph, pjp, pcps, pex)
                                        if pjp == 7:
                                            normalize(ph, pqs, pcps)
                                        pending[0] = None

                                # at jp==0 the pending block is (h-1, jp7): its
                                # normalize must precede any filler D matmul that
                                # reads the outnT rows it writes
                                if jp == 0:
                                    consume()
                                next(filler, None)
                                consume()
                                pending[0] = (h, qs, jp, cps, ex)
                        for _ in filler:
                            pass
                    ph, pqs, pjp, pcps, pex = pending[0]
                    emit_c(ph, pjp, pcps, pex)
                    normalize(ph, pqs, pcps)
                    for _ in d_filler(3):
                        pass

    nc.compile()
    return nc


def _get_nc():
    global _NC
    if _NC is None:
        _NC = _build()
    return _NC


def run(inputs, trace=False, trace_cores=None):
    from concourse.bass_utils import run_bass_kernel_spmd

    q = np.asarray(inputs["q"], np.float32)
    k = np.asarray(inputs["k"], np.float32)
    v = np.asarray(inputs["v"], np.float32)
    w_q = np.asarray(inputs["w_q"], np.float32)
    w_k = np.asarray(inputs["w_k"], np.float32)
    w_v = np.asarray(inputs["w_v"], np.float32)
    w_out = np.asarray(inputs["w_out"], np.float32)
    b_q = np.asarray(inputs["b_q"], np.float32)
    b_k = np.asarray(inputs["b_k"], np.float32)
    b_v = np.asarray(inputs["b_v"], np.float32)
    b_out = np.asarray(inputs["b_out"], np.float32)

    import ml_dtypes
    bf16 = ml_dtypes.bfloat16

    xT = {b: {} for b in range(B)}
    for b in range(B):
        xT[b]["qT"] = np.ascontiguousarray(q[b].T.astype(bf16))
        xT[b]["kT"] = np.ascontiguousarray(k[b].T.astype(bf16))
        xT[b]["vT"] = np.ascontiguousarray(v[b].T.astype(bf16))

    in_maps = []
    for c in range(N_CORES):
        b, hq = c // 4, c % 4
        rows = slice(hq * 256, (hq + 1) * 256)
        in_maps.append({
            "qT": xT[b]["qT"],
            "kT": xT[b]["kT"],
            "vT": xT[b]["vT"],
            "wq": np.ascontiguousarray(w_q[rows, :].T.astype(bf16)),
            "wk": np.ascontiguousarray(w_k[rows, :].T.astype(bf16)),
            "wv": np.ascontiguousarray(w_v[rows, :].T.astype(bf16)),
            "wo": np.ascontiguousarray(w_out[:, rows].T.astype(bf16)),
            "bq": np.ascontiguousarray(b_q[rows].reshape(256, 1)),
            "bk": np.ascontiguousarray(b_k[rows].reshape(256, 1)),
            "bv": np.ascontiguousarray(b_v[rows].reshape(1, 256)),
        })

    nc = _get_nc()
    res = run_bass_kernel_spmd(
        nc, in_maps, core_ids=list(range(N_CORES)), trace=trace,
        trace_cores=trace_cores,
    )
    full = np.zeros((B, S, D), np.float32)
    for c in range(N_CORES):
        full[c // 4] += np.asarray(res.results[c]["out"])
    full += b_out.reshape(1, 1, D)
    return full, res.exec_time_ns


def kernel(**inputs):
    return run(inputs, trace=False)[0]


# revision 13
# speedup vs baseline: 1.7858x; 1.2589x over previous
"""Multi-head attention on 8 Trainium2 cores.

Sharding: core c handles batch b = c // 4 and a quad of 4 heads
(hq = c % 4 -> heads 4*hq .. 4*hq+3) as two head-pairs of 64+64 = 128
partitions. w_q/w_k/w_v are split column-wise by head (tensor parallel),
w_out row-wise; per-batch partial outputs are summed on host.

Per-core pipeline (all matmuls bf16 in / fp32 PSUM out, 1 cycle/row):
  A: qhT/khT [128, 2048] = w.T-stationary matmuls over 8 d_model chunks;
     vh natural [j, dh] via vT-stationary matmuls, with a ones column
     appended per head (attn @ [vh | 1] yields softmax denominators).
  B: scoresT [j 128, q 512] = khT-chunk-stationary matmul (K=64), then
     ACT exp(x * 0.125) PSUM->SBUF.
  C: outT_aug [65, 512] accumulated over 16 j-chunks; row 64 = denom.
  Normalize: DVE reciprocal + gpsimd partition broadcast + DVE multiply.
  D: out [q 128, 1024] = outnT-stationary matmul over both head pairs
     (K=256), PSUM->SBUF->DMA.
"""

import numpy as np

B = 2
S = 2048
D = 1024
NH = 16
DH = 64
HEADS_PER_CORE = 4
N_CORES = 8

_NC = None


def _build():
    import concourse.bacc as bacc
    import concourse.tile as tile
    import concourse.mybir as mybir

    fp32 = mybir.dt.float32
    bf16 = mybir.dt.bfloat16
    add = mybir.AluOpType.add
    mult = mybir.AluOpType.mult
    Exp = mybir.ActivationFunctionType.Exp

    nc = bacc.Bacc("TRN2", target_bir_lowering=False)

    qT = nc.dram_tensor("qT", (D, S), bf16, kind="ExternalInput")
    kT = nc.dram_tensor("kT", (D, S), bf16, kind="ExternalInput")
    vT = nc.dram_tensor("vT", (D, S), bf16, kind="ExternalInput")
    wq = nc.dram_tensor("wq", (D, 256), bf16, kind="ExternalInput")
    wk = nc.dram_tensor("wk", (D, 256), bf16, kind="ExternalInput")
    wv = nc.dram_tensor("wv", (D, 256), bf16, kind="ExternalInput")
    wo = nc.dram_tensor("wo", (256, D), bf16, kind="ExternalInput")
    bq = nc.dram_tensor("bq", (256, 1), fp32, kind="ExternalInput")
    bk = nc.dram_tensor("bk", (256, 1), fp32, kind="ExternalInput")
    bv = nc.dram_tensor("bv", (1, 256), fp32, kind="ExternalInput")
    out = nc.dram_tensor("out", (S, D), fp32, kind="ExternalOutput")

    with tile.TileContext(nc) as tc:
        with tc.tile_pool(name="persist", bufs=1) as P:
            qhT = [P.tile((128, S), bf16, name=f"qhT{p}") for p in range(2)]
            khT = [P.tile((128, S), bf16, name=f"khT{p}") for p in range(2)]
            vh = [P.tile((128, 16 * 65), bf16, name=f"vh{h}") for h in range(4)]
            outnT = [P.tile((128, S), bf16, name=f"outnT{p}") for p in range(2)]
            wq_sb = P.tile((128, 8 * 256), bf16, name="wq_sb")
            wk_sb = P.tile((128, 8 * 256), bf16, name="wk_sb")
            wv_sb = P.tile((128, 8 * 256), bf16, name="wv_sb")
            wo_sb = [P.tile((128, D), bf16, name=f"wo_sb{p}") for p in range(2)]
            bq_sb = P.tile((128, 2), fp32, name="bq_sb")
            bk_sb = P.tile((128, 2), fp32, name="bk_sb")
            bv_row = P.tile((1, 256), fp32, name="bv_row")
            bv_bc = P.tile((128, 256), fp32, name="bv_bc")
            ones_f = P.tile((128, 16 * 65), fp32, name="ones_f")

            nc.gpsimd.memset(ones_f[:], 1.0)
            for h in range(4):
                nc.vector.tensor_scalar(
                    vh[h][:], ones_f[:], 1.0, None, op0=mybir.AluOpType.mult
                )

            for kc in range(8):
                nc.sync.dma_start(
                    wq_sb[:, kc * 256:(kc + 1) * 256], wq[kc * 128:(kc + 1) * 128, :]
                )
                nc.sync.dma_start(
                    wk_sb[:, kc * 256:(kc + 1) * 256], wk[kc * 128:(kc + 1) * 128, :]
                )
                nc.sync.dma_start(
                    wv_sb[:, kc * 256:(kc + 1) * 256], wv[kc * 128:(kc + 1) * 128, :]
                )
            for p in range(2):
                nc.sync.dma_start(wo_sb[p][:], wo[p * 128:(p + 1) * 128, :])
                nc.sync.dma_start(bq_sb[:, p:p + 1], bq[p * 128:(p + 1) * 128, :])
                nc.sync.dma_start(bk_sb[:, p:p + 1], bk[p * 128:(p + 1) * 128, :])
            nc.sync.dma_start(bv_row[:], bv[:])
            nc.gpsimd.partition_broadcast(bv_bc[:], bv_row[:])

            with tc.tile_pool(name="xin", bufs=2) as XP:

                def load_x(xdram, ns):
                    xt = XP.tile((128, 8 * 512), bf16, name="xt")
                    for kc in range(8):
                        nc.sync.dma_start(
                            xt[:, kc * 512:(kc + 1) * 512],
                            xdram[kc * 128:(kc + 1) * 128, ns * 512:(ns + 1) * 512],
                        )
                    return xt

                # ---- Upfront: khT (all), vh (all), qhT (qs=0 only) ----
                with tc.tile_pool(name="psA", bufs=2, space="PSUM") as PA, \
                     tc.tile_pool(name="psV", bufs=2, space="PSUM") as PV:

                    def proj_qk(xdram, w_sb, b_sb, dstT, ns):
                        xt = load_x(xdram, ns)
                        for p in range(2):
                            ps = PA.tile((128, 512), fp32, name="psa")
                            for kc in range(8):
                                nc.tensor.matmul(
                                    ps[:],
                                    w_sb[:, kc * 256 + p * 128:kc * 256 + (p + 1) * 128],
                                    xt[:, kc * 512:(kc + 1) * 512],
                                    start=(kc == 0),
                                    stop=(kc == 7),
                                )
                            nc.vector.tensor_scalar_add(
                                dstT[p][:, ns * 512:(ns + 1) * 512], ps[:], b_sb[:, p:p + 1]
                            )

                    for ns in range(4):
                        proj_qk(kT, wk_sb, bk_sb, khT, ns)
                    for ns in range(4):
                        xt = load_x(vT, ns)
                        for jj in range(4):
                            jc = ns * 4 + jj
                            ps = PV.tile((128, 256), fp32, name="psv")
                            for kc in range(8):
                                nc.tensor.matmul(
                                    ps[:],
                                    xt[:, kc * 512 + jj * 128:kc * 512 + (jj + 1) * 128],
                                    wv_sb[:, kc * 256:(kc + 1) * 256],
                                    start=(kc == 0),
                                    stop=(kc == 7),
                                )
                            for h in range(4):
                                nc.vector.scalar_tensor_tensor(
                                    vh[h][:, jc * 65:jc * 65 + 64],
                                    ps[:, h * 64:(h + 1) * 64],
                                    1.0,
                                    bv_bc[:, h * 64:(h + 1) * 64],
                                    op0=mult,
                                    op1=add,
                                )
                    proj_qk(qT, wq_sb, bq_sb, qhT, 0)

                # ---- Main loop: B+exp with interleaved filler (qh proj / D) ----
                with tc.tile_pool(name="psS", bufs=2, space="PSUM") as PS, \
                     tc.tile_pool(name="psC", bufs=2, space="PSUM") as PC, \
                     tc.tile_pool(name="psD", bufs=1, space="PSUM") as PD, \
                     tc.tile_pool(name="psF", bufs=1, space="PSUM") as PF, \
                     tc.tile_pool(name="expP", bufs=3) as EP, \
                     tc.tile_pool(name="nrm", bufs=2) as NP, \
                     tc.tile_pool(name="outP", bufs=2) as OP:

                    def qh_filler(ns):
                        xt = load_x(qT, ns)
                        for p in range(2):
                            ps = PF.tile((128, 512), fp32, name="psf")
                            for kc in range(8):
                                nc.tensor.matmul(
                                    ps[:],
                                    wq_sb[:, kc * 256 + p * 128:kc * 256 + (p + 1) * 128],
                                    xt[:, kc * 512:(kc + 1) * 512],
                                    start=(kc == 0),
                                    stop=(kc == 7),
                                )
                                yield
                            nc.vector.tensor_scalar_add(
                                qhT[p][:, ns * 512:(ns + 1) * 512], ps[:], bq_sb[:, p:p + 1]
                            )

                    def d_filler(qs):
                        for qq in range(4):
                            qc0 = qs * 512 + qq * 128
                            osb = OP.tile((128, D), fp32, name="osb")
                            for nsd in range(2):
                                dps = PD.tile((128, 512), fp32, name="dps")
                                for p in range(2):
                                    nc.tensor.matmul(
                                        dps[:],
                                        outnT[p][:, qc0:qc0 + 128],
                                        wo_sb[p][:, nsd * 512:(nsd + 1) * 512],
                                        start=(p == 0),
                                        stop=(p == 1),
                                    )
                                    yield
                                nc.vector.tensor_scalar_add(
                                    osb[:, nsd * 512:(nsd + 1) * 512], dps[:], 0.0
                                )
                            nc.sync.dma_start(out[qc0:qc0 + 128, :], osb[:])

                    def chain(*gens):
                        for g in gens:
                            yield from g

                    def emit_c(ph, pjp, pcps, pex):
                        for half in range(2):
                            pjc = pjp * 2 + half
                            nc.tensor.matmul(
                                pcps[:],
                                vh[ph][:, pjc * 65:(pjc + 1) * 65],
                                pex[:, half * 512:(half + 1) * 512],
                                start=(pjc == 0),
                                stop=(pjc == 15),
                            )

                    def normalize(ph, pqs, pcps):
                        p, off = ph // 2, (ph % 2) * 64
                        den = NP.tile((1, 512), fp32, name="den")
                        # approx_fast requires SBUF input; copy denom row off PSUM first
                        nc.vector.tensor_scalar_add(den[:], pcps[64:65, :], 0.0)
                        rec = NP.tile((1, 512), fp32, name="rec")
                        nc.vector.reciprocal_approx_fast(rec[:], den[:])
                        rbc = NP.tile((64, 512), fp32, name="rbc")
                        nc.gpsimd.partition_broadcast(rbc[:], rec[:])
                        nc.vector.scalar_tensor_tensor(
                            outnT[p][off:off + 64, pqs * 512:(pqs + 1) * 512],
                            pcps[0:64, :],
                            1.0,
                            rbc[:],
                            op0=mult,
                            op1=mult,
                        )

                    pending = [None]
                    for qs in range(4):
                        if qs == 0:
                            filler = chain(qh_filler(1), qh_filler(2))
                        elif qs == 1:
                            filler = chain(qh_filler(3), d_filler(0))
                        elif qs == 2:
                            filler = d_filler(1)
                        else:
                            filler = d_filler(2)
                        for h in range(4):
                            p, off = h // 2, (h % 2) * 64
                            cps = PC.tile((65, 512), fp32, name="cps")
                            for jp in range(8):
                                sps = PS.tile((128, 1024), fp32, name="sps")
                                for half in range(2):
                                    jc = jp * 2 + half
                                    nc.tensor.matmul(
                                        sps[:, half * 512:(half + 1) * 512],
                                        khT[p][off:off + 64, jc * 128:(jc + 1) * 128],
                                        qhT[p][off:off + 64, qs * 512:(qs + 1) * 512],
                                        start=True,
                                        stop=True,
                                    )
                                ex = EP.tile((128, 1024), bf16, name="ex")
                                nc.scalar.activation(ex[:], sps[:], Exp, bias=0.0, scale=0.125)

                                def consume():
                                    if pending[0] is not None:
                                        ph, pqs, pjp, pcps, pex = pending[0]
                                        emit_c(ph, pjp, pcps, pex)
                                        if pjp == 7:
                                            normalize(ph, pqs, pcps)
                                        pending[0] = None

                                # at jp==0 the pending block is (h-1, jp7): its
                                # normalize must precede any filler D matmul that
                                # reads the outnT rows it writes
                                if jp == 0:
                                    consume()
                                next(filler, None)
                                consume()
                                pending[0] = (h, qs, jp, cps, ex)
                        for _ in filler:
                            pass
                    ph, pqs, pjp, pcps, pex = pending[0]
                    emit_c(ph, pjp, pcps, pex)
                    normalize(ph, pqs, pcps)
                    for _ in d_filler(3):
                        pass

    nc.compile()
    return nc


def _get_nc():
    global _NC
    if _NC is None:
        _NC = _build()
    return _NC


def run(inputs, trace=False, trace_cores=None):
    from concourse.bass_utils import run_bass_kernel_spmd

    q = np.asarray(inputs["q"], np.float32)
    k = np.asarray(inputs["k"], np.float32)
    v = np.asarray(inputs["v"], np.float32)
    w_q = np.asarray(inputs["w_q"], np.float32)
    w_k = np.asarray(inputs["w_k"], np.float32)
    w_v = np.asarray(inputs["w_v"], np.float32)
    w_out = np.asarray(inputs["w_out"], np.float32)
    b_q = np.asarray(inputs["b_q"], np.float32)
    b_k = np.asarray(inputs["b_k"], np.float32)
    b_v = np.asarray(inputs["b_v"], np.float32)
    b_out = np.asarray(inputs["b_out"], np.float32)

    import ml_dtypes
    bf16 = ml_dtypes.bfloat16

    xT = {b: {} for b in range(B)}
    for b in range(B):
        xT[b]["qT"] = np.ascontiguousarray(q[b].T.astype(bf16))
        xT[b]["kT"] = np.ascontiguousarray(k[b].T.astype(bf16))
        xT[b]["vT"] = np.ascontiguousarray(v[b].T.astype(bf16))

    in_maps = []
    for c in range(N_CORES):
        b, hq = c // 4, c % 4
        rows = slice(hq * 256, (hq + 1) * 256)
        in_maps.append({
            "qT": xT[b]["qT"],
            "kT": xT[b]["kT"],
            "vT": xT[b]["vT"],
            "wq": np.ascontiguousarray(w_q[rows, :].T.astype(bf16)),
            "wk": np.ascontiguousarray(w_k[rows, :].T.astype(bf16)),
            "wv": np.ascontiguousarray(w_v[rows, :].T.astype(bf16)),
            "wo": np.ascontiguousarray(w_out[:, rows].T.astype(bf16)),
            "bq": np.ascontiguousarray(b_q[rows].reshape(256, 1)),
            "bk": np.ascontiguousarray(b_k[rows].reshape(256, 1)),
            "bv": np.ascontiguousarray(b_v[rows].reshape(1, 256)),
        })

    nc = _get_nc()
    res = run_bass_kernel_spmd(
        nc, in_maps, core_ids=list(range(N_CORES)), trace=trace,
        trace_cores=trace_cores,
    )
    full = np.zeros((B, S, D), np.float32)
    for c in range(N_CORES):
        full[c // 4] += np.asarray(res.results[c]["out"])
    full += b_out.reshape(1, 1, D)
    return full, res.exec_time_ns


def kernel(**inputs):
    return run(inputs, trace=False)[0]


# revision 19
# speedup vs baseline: 1.8291x; 1.0242x over previous
"""Multi-head attention on 8 Trainium2 cores.

Sharding: core c handles batch b = c // 4 and a quad of 4 heads
(hq = c % 4 -> heads 4*hq .. 4*hq+3) as two head-pairs of 64+64 = 128
partitions. w_q/w_k/w_v are split column-wise by head (tensor parallel),
w_out row-wise; per-batch partial outputs are summed on host.

Per-core pipeline (all matmuls bf16 in / fp32 PSUM out, 1 cycle/row):
  A: qhT/khT [128, 2048] = w.T-stationary matmuls over 8 d_model chunks;
     vh natural [j, dh] via vT-stationary matmuls, with a ones column
     appended per head (attn @ [vh | 1] yields softmax denominators).
  B: scoresT [j 128, q 512] = khT-chunk-stationary matmul (K=64), then
     ACT exp(x * 0.125) PSUM->SBUF.
  C: outT_aug [65, 512] accumulated over 16 j-chunks; row 64 = denom.
  Normalize: DVE reciprocal + gpsimd partition broadcast + DVE multiply.
  D: out [q 128, 1024] = outnT-stationary matmul over both head pairs
     (K=256), PSUM->SBUF->DMA.
"""

import numpy as np

B = 2
S = 2048
D = 1024
NH = 16
DH = 64
HEADS_PER_CORE = 4
N_CORES = 8

_NC = None


def _build():
    import concourse.bacc as bacc
    import concourse.tile as tile
    import concourse.mybir as mybir

    fp32 = mybir.dt.float32
    bf16 = mybir.dt.bfloat16
    add = mybir.AluOpType.add
    mult = mybir.AluOpType.mult
    Exp = mybir.ActivationFunctionType.Exp

    nc = bacc.Bacc("TRN2", target_bir_lowering=False)

    qT = nc.dram_tensor("qT", (D, S), bf16, kind="ExternalInput")
    kT = nc.dram_tensor("kT", (D, S), bf16, kind="ExternalInput")
    vT = nc.dram_tensor("vT", (D, S), bf16, kind="ExternalInput")
    wq = nc.dram_tensor("wq", (D, 256), bf16, kind="ExternalInput")
    wk = nc.dram_tensor("wk", (D, 256), bf16, kind="ExternalInput")
    wv = nc.dram_tensor("wv", (D, 256), bf16, kind="ExternalInput")
    wo = nc.dram_tensor("wo", (256, D), bf16, kind="ExternalInput")
    bq = nc.dram_tensor("bq", (256, 1), fp32, kind="ExternalInput")
    bk = nc.dram_tensor("bk", (256, 1), fp32, kind="ExternalInput")
    bv = nc.dram_tensor("bv", (1, 256), fp32, kind="ExternalInput")
    out = nc.dram_tensor("out", (S, D), fp32, kind="ExternalOutput")

    with tile.TileContext(nc) as tc:
        with tc.tile_pool(name="persist", bufs=1) as P:
            qhT = [P.tile((128, S), bf16, name=f"qhT{p}") for p in range(2)]
            khT = [P.tile((128, S), bf16, name=f"khT{p}") for p in range(2)]
            vh = [P.tile((128, 16 * 65), bf16, name=f"vh{h}") for h in range(4)]
            outnT = [P.tile((128, S), bf16, name=f"outnT{p}") for p in range(2)]
            wq_sb = P.tile((128, 8 * 256), bf16, name="wq_sb")
            wk_sb = P.tile((128, 8 * 256), bf16, name="wk_sb")
            wv_sb = P.tile((128, 8 * 256), bf16, name="wv_sb")
            wo_sb = [P.tile((128, D), bf16, name=f"wo_sb{p}") for p in range(2)]
            bq_sb = P.tile((128, 2), fp32, name="bq_sb")
            bk_sb = P.tile((128, 2), fp32, name="bk_sb")
            bv_row = P.tile((1, 256), fp32, name="bv_row")
            bv_bc = P.tile((128, 256), fp32, name="bv_bc")
            ones_f = P.tile((128, 16 * 65), fp32, name="ones_f")

            nc.gpsimd.memset(ones_f[:], 1.0)
            for h in range(4):
                nc.vector.tensor_scalar(
                    vh[h][:], ones_f[:], 1.0, None, op0=mybir.AluOpType.mult
                )

            # weights/biases on the ACT hwdge queue (ACT idle during stage A)
            # so they don't sit in front of the x-chunk loads on the SP queue
            for kc in range(8):
                nc.scalar.dma_start(
                    wk_sb[:, kc * 256:(kc + 1) * 256], wk[kc * 128:(kc + 1) * 128, :]
                )
            for kc in range(8):
                nc.scalar.dma_start(
                    wv_sb[:, kc * 256:(kc + 1) * 256], wv[kc * 128:(kc + 1) * 128, :]
                )
            for kc in range(8):
                nc.scalar.dma_start(
                    wq_sb[:, kc * 256:(kc + 1) * 256], wq[kc * 128:(kc + 1) * 128, :]
                )
            for p in range(2):
                nc.scalar.dma_start(wo_sb[p][:], wo[p * 128:(p + 1) * 128, :])
                nc.scalar.dma_start(bq_sb[:, p:p + 1], bq[p * 128:(p + 1) * 128, :])
                nc.scalar.dma_start(bk_sb[:, p:p + 1], bk[p * 128:(p + 1) * 128, :])
            nc.scalar.dma_start(bv_row[:], bv[:])
            nc.gpsimd.partition_broadcast(bv_bc[:], bv_row[:])

            with tc.tile_pool(name="xin", bufs=2) as XP:

                def load_x(xdram, ns, eng=None):
                    eng = eng or nc.sync
                    xt = XP.tile((128, 8 * 512), bf16, name="xt")
                    for kc in range(8):
                        eng.dma_start(
                            xt[:, kc * 512:(kc + 1) * 512],
                            xdram[kc * 128:(kc + 1) * 128, ns * 512:(ns + 1) * 512],
                        )
                    return xt

                # ---- Upfront: khT (all), vh (all), qhT (qs=0 only) ----
                with tc.tile_pool(name="psA", bufs=2, space="PSUM") as PA, \
                     tc.tile_pool(name="psV", bufs=2, space="PSUM") as PV:

                    def proj_qk(xdram, w_sb, b_sb, dstT, ns, eng=None):
                        xt = load_x(xdram, ns, eng)
                        for p in range(2):
                            ps = PA.tile((128, 512), fp32, name="psa")
                            for kc in range(8):
                                nc.tensor.matmul(
                                    ps[:],
                                    w_sb[:, kc * 256 + p * 128:kc * 256 + (p + 1) * 128],
                                    xt[:, kc * 512:(kc + 1) * 512],
                                    start=(kc == 0),
                                    stop=(kc == 7),
                                )
                            nc.vector.tensor_scalar_add(
                                dstT[p][:, ns * 512:(ns + 1) * 512], ps[:], b_sb[:, p:p + 1]
                            )

                    for ns in range(4):
                        proj_qk(kT, wk_sb, bk_sb, khT, ns,
                                nc.scalar if ns % 2 else nc.sync)
                    for ns in range(4):
                        xt = load_x(vT, ns, nc.scalar if ns % 2 else nc.sync)
                        for jj in range(4):
                            jc = ns * 4 + jj
                            ps = PV.tile((128, 256), fp32, name="psv")
                            for kc in range(8):
                                nc.tensor.matmul(
                                    ps[:],
                                    xt[:, kc * 512 + jj * 128:kc * 512 + (jj + 1) * 128],
                                    wv_sb[:, kc * 256:(kc + 1) * 256],
                                    start=(kc == 0),
                                    stop=(kc == 7),
                                )
                            for h in range(4):
                                nc.vector.scalar_tensor_tensor(
                                    vh[h][:, jc * 65:jc * 65 + 64],
                                    ps[:, h * 64:(h + 1) * 64],
                                    1.0,
                                    bv_bc[:, h * 64:(h + 1) * 64],
                                    op0=mult,
                                    op1=add,
                                )
                    proj_qk(qT, wq_sb, bq_sb, qhT, 0)

                # ---- Main loop: B+exp with interleaved filler (qh proj / D) ----
                with tc.tile_pool(name="psS", bufs=2, space="PSUM") as PS, \
                     tc.tile_pool(name="psC", bufs=2, space="PSUM") as PC, \
                     tc.tile_pool(name="psD", bufs=1, space="PSUM") as PD, \
                     tc.tile_pool(name="psF", bufs=1, space="PSUM") as PF, \
                     tc.tile_pool(name="expP", bufs=3) as EP, \
                     tc.tile_pool(name="nrm", bufs=2) as NP, \
                     tc.tile_pool(name="outP", bufs=2) as OP:

                    def qh_filler(ns):
                        xt = load_x(qT, ns)
                        for p in range(2):
                            ps = PF.tile((128, 512), fp32, name="psf")
                            for kc in range(8):
                                nc.tensor.matmul(
                                    ps[:],
                                    wq_sb[:, kc * 256 + p * 128:kc * 256 + (p + 1) * 128],
                                    xt[:, kc * 512:(kc + 1) * 512],
                                    start=(kc == 0),
                                    stop=(kc == 7),
                                )
                                yield
                            nc.vector.tensor_scalar_add(
                                qhT[p][:, ns * 512:(ns + 1) * 512], ps[:], bq_sb[:, p:p + 1]
                            )

                    def d_filler(qs, tail=False):
                        for qq in range(4):
                            qc0 = qs * 512 + qq * 128
                            osb = OP.tile((128, D), fp32, name="osb")
                            for nsd in range(2):
                                dps = PD.tile((128, 512), fp32, name="dps")
                                for p in range(2):
                                    nc.tensor.matmul(
                                        dps[:],
                                        outnT[p][:, qc0:qc0 + 128],
                                        wo_sb[p][:, nsd * 512:(nsd + 1) * 512],
                                        start=(p == 0),
                                        stop=(p == 1),
                                    )
                                    yield
                                nc.vector.tensor_scalar_add(
                                    osb[:, nsd * 512:(nsd + 1) * 512], dps[:], 0.0
                                )
                            # at the tail ACT is idle: split final out DMAs
                            # across both hwdge queues to halve drain time
                            eng = nc.scalar if (tail and qq % 2) else nc.sync
                            eng.dma_start(out[qc0:qc0 + 128, :], osb[:])

                    def chain(*gens):
                        for g in gens:
                            yield from g

                    def emit_c(ph, pjp, pcps, pex):
                        for half in range(2):
                            pjc = pjp * 2 + half
                            nc.tensor.matmul(
                                pcps[:],
                                vh[ph][:, pjc * 65:(pjc + 1) * 65],
                                pex[:, half * 512:(half + 1) * 512],
                                start=(pjc == 0),
                                stop=(pjc == 15),
                            )

                    def normalize(ph, pqs, pcps):
                        p, off = ph // 2, (ph % 2) * 64
                        den = NP.tile((1, 512), fp32, name="den")
                        # approx_fast requires SBUF input; copy denom row off PSUM first
                        nc.vector.tensor_scalar_add(den[:], pcps[64:65, :], 0.0)
                        rec = NP.tile((1, 512), fp32, name="rec")
                        nc.vector.reciprocal_approx_fast(rec[:], den[:])
                        rbc = NP.tile((64, 512), fp32, name="rbc")
                        nc.gpsimd.partition_broadcast(rbc[:], rec[:])
                        nc.vector.scalar_tensor_tensor(
                            outnT[p][off:off + 64, pqs * 512:(pqs + 1) * 512],
                            pcps[0:64, :],
                            1.0,
                            rbc[:],
                            op0=mult,
                            op1=mult,
                        )

                    pending = [None]
                    for qs in range(4):
                        if qs == 0:
                            filler = chain(qh_filler(1), qh_filler(2))
                        elif qs == 1:
                            filler = chain(qh_filler(3), d_filler(0))
                        elif qs == 2:
                            filler = d_filler(1)
                        else:
                            filler = d_filler(2)
                        for h in range(4):
                            p, off = h // 2, (h % 2) * 64
                            cps = PC.tile((65, 512), fp32, name="cps")
                            for jp in range(8):
                                sps = PS.tile((128, 1024), fp32, name="sps")
                                for half in range(2):
                                    jc = jp * 2 + half
                                    nc.tensor.matmul(
                                        sps[:, half * 512:(half + 1) * 512],
                                        khT[p][off:off + 64, jc * 128:(jc + 1) * 128],
                                        qhT[p][off:off + 64, qs * 512:(qs + 1) * 512],
                                        start=True,
                                        stop=True,
                                    )
                                ex = EP.tile((128, 1024), bf16, name="ex")
                                nc.scalar.activation(ex[:], sps[:], Exp, bias=0.0, scale=0.125)

                                def consume():
                                    if pending[0] is not None:
                                        ph, pqs, pjp, pcps, pex = pending[0]
                                        emit_c(ph, pjp, pcps, pex)
                                        if pjp == 7:
                                            normalize(ph, pqs, pcps)
                                        pending[0] = None

                                # at jp==0 the pending block is (h-1, jp7): its
                                # normalize must precede any filler D matmul that
                                # reads the outnT rows it writes
                                if jp == 0:
                                    consume()
                                next(filler, None)
                                consume()
                                pending[0] = (h, qs, jp, cps, ex)
                        for _ in filler:
                            pass
                    ph, pqs, pjp, pcps, pex = pending[0]
                    emit_c(ph, pjp, pcps, pex)
                    normalize(ph, pqs, pcps)
                    for _ in d_filler(3, tail=True):
                        pass

    nc.compile()
    return nc


def _get_nc():
    global _NC
    if _NC is None:
        _NC = _build()
    return _NC


def run(inputs, trace=False, trace_cores=None):
    from concourse.bass_utils import run_bass_kernel_spmd

    q = np.asarray(inputs["q"], np.float32)
    k = np.asarray(inputs["k"], np.float32)
    v = np.asarray(inputs["v"], np.float32)
    w_q = np.asarray(inputs["w_q"], np.float32)
    w_k = np.asarray(inputs["w_k"], np.float32)
    w_v = np.asarray(inputs["w_v"], np.float32)
    w_out = np.asarray(inputs["w_out"], np.float32)
    b_q = np.asarray(inputs["b_q"], np.float32)
    b_k = np.asarray(inputs["b_k"], np.float32)
    b_v = np.asarray(inputs["b_v"], np.float32)
    b_out = np.asarray(inputs["b_out"], np.float32)

    import ml_dtypes
    bf16 = ml_dtypes.bfloat16

    xT = {b: {} for b in range(B)}
    for b in range(B):
        xT[b]["qT"] = np.ascontiguousarray(q[b].T.astype(bf16))
        xT[b]["kT"] = np.ascontiguousarray(k[b].T.astype(bf16))
        xT[b]["vT"] = np.ascontiguousarray(v[b].T.astype(bf16))

    in_maps = []
    for c in range(N_CORES):
        b, hq = c // 4, c % 4
        rows = slice(hq * 256, (hq + 1) * 256)
        in_maps.append({
            "qT": xT[b]["qT"],
            "kT": xT[b]["kT"],
            "vT": xT[b]["vT"],
            "wq": np.ascontiguousarray(w_q[rows, :].T.astype(bf16)),
            "wk": np.ascontiguousarray(w_k[rows, :].T.astype(bf16)),
            "wv": np.ascontiguousarray(w_v[rows, :].T.astype(bf16)),
            "wo": np.ascontiguousarray(w_out[:, rows].T.astype(bf16)),
            "bq": np.ascontiguousarray(b_q[rows].reshape(256, 1)),
            "bk": np.ascontiguousarray(b_k[rows].reshape(256, 1)),
            "bv": np.ascontiguousarray(b_v[rows].reshape(1, 256)),
        })

    nc = _get_nc()
    res = run_bass_kernel_spmd(
        nc, in_maps, core_ids=list(range(N_CORES)), trace=trace,
        trace_cores=trace_cores,
    )
    full = np.zeros((B, S, D), np.float32)
    for c in range(N_CORES):
        full[c // 4] += np.asarray(res.results[c]["out"])
    full += b_out.reshape(1, 1, D)
    return full, res.exec_time_ns


def kernel(**inputs):
    return run(inputs, trace=False)[0]


# revision 26
# speedup vs baseline: 1.8552x; 1.0143x over previous
"""Multi-head attention on 8 Trainium2 cores.

Sharding: core c handles batch b = c // 4 and a quad of 4 heads
(hq = c % 4 -> heads 4*hq .. 4*hq+3) as two head-pairs of 64+64 = 128
partitions. w_q/w_k/w_v are split column-wise by head (tensor parallel),
w_out row-wise; per-batch partial outputs are summed on host.

Per-core pipeline (all matmuls bf16 in / fp32 PSUM out, 1 cycle/row):
  A: qhT/khT [128, 2048] = w.T-stationary matmuls over 8 d_model chunks;
     vh natural [j, dh] via vT-stationary matmuls, with a ones column
     appended per head (attn @ [vh | 1] yields softmax denominators).
  B: scoresT [j 128, q 512] = khT-chunk-stationary matmul (K=64), then
     ACT exp(x * 0.125) PSUM->SBUF.
  C: outT_aug [65, 512] accumulated over 16 j-chunks; row 64 = denom.
  Normalize: DVE reciprocal + gpsimd partition broadcast + DVE multiply.
  D: out [q 128, 1024] = outnT-stationary matmul over both head pairs
     (K=256), PSUM->SBUF->DMA.
"""

import numpy as np

B = 2
S = 2048
D = 1024
NH = 16
DH = 64
HEADS_PER_CORE = 4
N_CORES = 8

_NC = None


def _build():
    import concourse.bacc as bacc
    import concourse.tile as tile
    import concourse.mybir as mybir

    fp32 = mybir.dt.float32
    bf16 = mybir.dt.bfloat16
    add = mybir.AluOpType.add
    mult = mybir.AluOpType.mult
    Exp = mybir.ActivationFunctionType.Exp

    nc = bacc.Bacc("TRN2", target_bir_lowering=False)

    qT = nc.dram_tensor("qT", (D, S), bf16, kind="ExternalInput")
    kT = nc.dram_tensor("kT", (D, S), bf16, kind="ExternalInput")
    vT = nc.dram_tensor("vT", (D, S), bf16, kind="ExternalInput")
    wq = nc.dram_tensor("wq", (D, 256), bf16, kind="ExternalInput")
    wk = nc.dram_tensor("wk", (D, 256), bf16, kind="ExternalInput")
    wv = nc.dram_tensor("wv", (D, 256), bf16, kind="ExternalInput")
    wo = nc.dram_tensor("wo", (256, D), bf16, kind="ExternalInput")
    bq = nc.dram_tensor("bq", (256, 1), fp32, kind="ExternalInput")
    bk = nc.dram_tensor("bk", (256, 1), fp32, kind="ExternalInput")
    bv = nc.dram_tensor("bv", (1, 256), fp32, kind="ExternalInput")
    out = nc.dram_tensor("out", (S, D), fp32, kind="ExternalOutput")

    with tile.TileContext(nc) as tc:
        with tc.tile_pool(name="persist", bufs=1) as P:
            qhT = [P.tile((128, S), bf16, name=f"qhT{p}") for p in range(2)]
            khT = [P.tile((128, S), bf16, name=f"khT{p}") for p in range(2)]
            vh = [P.tile((128, 16 * 65), bf16, name=f"vh{h}") for h in range(4)]
            outnT = [P.tile((128, S), bf16, name=f"outnT{p}") for p in range(2)]
            wq_sb = P.tile((128, 8 * 256), bf16, name="wq_sb")
            wk_sb = P.tile((128, 8 * 256), bf16, name="wk_sb")
            wv_sb = P.tile((128, 8 * 256), bf16, name="wv_sb")
            wo_sb = [P.tile((128, D), bf16, name=f"wo_sb{p}") for p in range(2)]
            bq_sb = P.tile((128, 2), fp32, name="bq_sb")
            bk_sb = P.tile((128, 2), fp32, name="bk_sb")
            bv_row = P.tile((1, 256), fp32, name="bv_row")
            bv_bc = P.tile((128, 256), fp32, name="bv_bc")
            ones_f = P.tile((128, 16 * 65), fp32, name="ones_f")

            nc.gpsimd.memset(ones_f[:], 1.0)
            for h in range(4):
                nc.vector.tensor_scalar(
                    vh[h][:], ones_f[:], 1.0, None, op0=mybir.AluOpType.mult
                )

            # weights/biases on the ACT hwdge queue (ACT idle during stage A),
            # emitted just-in-time so each weight lands right before the
            # projection that consumes it, keeping both queues feeding PE
            for p in range(2):
                nc.scalar.dma_start(bq_sb[:, p:p + 1], bq[p * 128:(p + 1) * 128, :])
                nc.scalar.dma_start(bk_sb[:, p:p + 1], bk[p * 128:(p + 1) * 128, :])
            nc.scalar.dma_start(bv_row[:], bv[:])
            for kc in range(8):
                nc.scalar.dma_start(
                    wk_sb[:, kc * 256:(kc + 1) * 256], wk[kc * 128:(kc + 1) * 128, :]
                )
            nc.gpsimd.partition_broadcast(bv_bc[:], bv_row[:])

            with tc.tile_pool(name="xin", bufs=2) as XP:

                def load_x(xdram, ns, eng=None):
                    eng = eng or nc.sync
                    xt = XP.tile((128, 8 * 512), bf16, name="xt")
                    for kc in range(8):
                        eng.dma_start(
                            xt[:, kc * 512:(kc + 1) * 512],
                            xdram[kc * 128:(kc + 1) * 128, ns * 512:(ns + 1) * 512],
                        )
                    return xt

                # ---- Upfront: khT (all), vh (all), qhT (qs=0 only) ----
                with tc.tile_pool(name="psA", bufs=2, space="PSUM") as PA, \
                     tc.tile_pool(name="psV", bufs=2, space="PSUM") as PV:

                    def proj_qk(xdram, w_sb, b_sb, dstT, ns, eng=None):
                        xt = load_x(xdram, ns, eng)
                        for p in range(2):
                            ps = PA.tile((128, 512), fp32, name="psa")
                            for kc in range(8):
                                nc.tensor.matmul(
                                    ps[:],
                                    w_sb[:, kc * 256 + p * 128:kc * 256 + (p + 1) * 128],
                                    xt[:, kc * 512:(kc + 1) * 512],
                                    start=(kc == 0),
                                    stop=(kc == 7),
                                )
                            nc.vector.tensor_scalar_add(
                                dstT[p][:, ns * 512:(ns + 1) * 512], ps[:], b_sb[:, p:p + 1]
                            )

                    for ns in range(4):
                        proj_qk(kT, wk_sb, bk_sb, khT, ns,
                                nc.scalar if ns % 2 else nc.sync)
                    for kc in range(8):
                        nc.scalar.dma_start(
                            wv_sb[:, kc * 256:(kc + 1) * 256],
                            wv[kc * 128:(kc + 1) * 128, :],
                        )
                    for ns in range(4):
                        xt = load_x(vT, ns, nc.scalar if ns % 2 else nc.sync)
                        for jj in range(4):
                            jc = ns * 4 + jj
                            ps = PV.tile((128, 256), fp32, name="psv")
                            for kc in range(8):
                                nc.tensor.matmul(
                                    ps[:],
                                    xt[:, kc * 512 + jj * 128:kc * 512 + (jj + 1) * 128],
                                    wv_sb[:, kc * 256:(kc + 1) * 256],
                                    start=(kc == 0),
                                    stop=(kc == 7),
                                )
                            for h in range(4):
                                nc.vector.scalar_tensor_tensor(
                                    vh[h][:, jc * 65:jc * 65 + 64],
                                    ps[:, h * 64:(h + 1) * 64],
                                    1.0,
                                    bv_bc[:, h * 64:(h + 1) * 64],
                                    op0=mult,
                                    op1=add,
                                )
                    for kc in range(8):
                        nc.scalar.dma_start(
                            wq_sb[:, kc * 256:(kc + 1) * 256],
                            wq[kc * 128:(kc + 1) * 128, :],
                        )
                    proj_qk(qT, wq_sb, bq_sb, qhT, 0)
                    for p in range(2):
                        nc.scalar.dma_start(wo_sb[p][:], wo[p * 128:(p + 1) * 128, :])

                # ---- Main loop: B+exp with interleaved filler (qh proj / D) ----
                with tc.tile_pool(name="psS", bufs=2, space="PSUM") as PS, \
                     tc.tile_pool(name="psC", bufs=2, space="PSUM") as PC, \
                     tc.tile_pool(name="psDF", bufs=2, space="PSUM") as PD, \
                     tc.tile_pool(name="expP", bufs=3) as EP, \
                     tc.tile_pool(name="nrm", bufs=2) as NP, \
                     tc.tile_pool(name="outP", bufs=2) as OP:

                    def qh_filler(ns):
                        xt = load_x(qT, ns)
                        for p in range(2):
                            ps = PD.tile((128, 512), fp32, name="dps")
                            for kc in range(8):
                                nc.tensor.matmul(
                                    ps[:],
                                    wq_sb[:, kc * 256 + p * 128:kc * 256 + (p + 1) * 128],
                                    xt[:, kc * 512:(kc + 1) * 512],
                                    start=(kc == 0),
                                    stop=(kc == 7),
                                )
                                yield
                            nc.vector.tensor_scalar_add(
                                qhT[p][:, ns * 512:(ns + 1) * 512], ps[:], bq_sb[:, p:p + 1]
                            )

                    def d_filler(qs, tail=False):
                        for qq in range(4):
                            qc0 = qs * 512 + qq * 128
                            osb = OP.tile((128, D), fp32, name="osb")
                            for nsd in range(2):
                                dps = PD.tile((128, 512), fp32, name="dps")
                                for p in range(2):
                                    nc.tensor.matmul(
                                        dps[:],
                                        outnT[p][:, qc0:qc0 + 128],
                                        wo_sb[p][:, nsd * 512:(nsd + 1) * 512],
                                        start=(p == 0),
                                        stop=(p == 1),
                                    )
                                    yield
                                nc.vector.tensor_scalar_add(
                                    osb[:, nsd * 512:(nsd + 1) * 512], dps[:], 0.0
                                )
                            # at the tail ACT is idle: split final out DMAs
                            # across both hwdge queues to halve drain time
                            eng = nc.scalar if (tail and qq % 2) else nc.sync
                            eng.dma_start(out[qc0:qc0 + 128, :], osb[:])

                    def chain(*gens):
                        for g in gens:
                            yield from g

                    def emit_c(ph, pjp, pcps, pex):
                        for half in range(2):
                            pjc = pjp * 2 + half
                            nc.tensor.matmul(
                                pcps[:],
                                vh[ph][:, pjc * 65:(pjc + 1) * 65],
                                pex[:, half * 512:(half + 1) * 512],
                                start=(pjc == 0),
                                stop=(pjc == 15),
                            )

                    def normalize(ph, pqs, pcps):
                        p, off = ph // 2, (ph % 2) * 64
                        den = NP.tile((1, 512), fp32, name="den")
                        # approx_fast requires SBUF input; copy denom row off PSUM first
                        nc.vector.tensor_scalar_add(den[:], pcps[64:65, :], 0.0)
                        rec = NP.tile((1, 512), fp32, name="rec")
                        nc.vector.reciprocal_approx_fast(rec[:], den[:])
                        rbc = NP.tile((64, 512), fp32, name="rbc")
                        nc.gpsimd.partition_broadcast(rbc[:], rec[:])
                        nc.vector.scalar_tensor_tensor(
                            outnT[p][off:off + 64, pqs * 512:(pqs + 1) * 512],
                            pcps[0:64, :],
                            1.0,
                            rbc[:],
                            op0=mult,
                            op1=mult,
                        )

                    pending = [None]
                    for qs in range(4):
                        if qs == 0:
                            filler = chain(qh_filler(1), qh_filler(2))
                        elif qs == 1:
                            filler = chain(qh_filler(3), d_filler(0))
                        elif qs == 2:
                            filler = d_filler(1)
                        else:
                            filler = d_filler(2)
                        for h in range(4):
                            p, off = h // 2, (h % 2) * 64
                            cps = PC.tile((65, 512), fp32, name="cps")
                            for jp in range(8):
                                sps = PS.tile((128, 1024), fp32, name="sps")
                                for half in range(2):
                                    jc = jp * 2 + half
                                    nc.tensor.matmul(
                                        sps[:, half * 512:(half + 1) * 512],
                                        khT[p][off:off + 64, jc * 128:(jc + 1) * 128],
                                        qhT[p][off:off + 64, qs * 512:(qs + 1) * 512],
                                        start=True,
                                        stop=True,
                                    )
                                ex = EP.tile((128, 1024), bf16, name="ex")
                                nc.scalar.activation(ex[:], sps[:], Exp, bias=0.0, scale=0.125)

                                def consume():
                                    if pending[0] is not None:
                                        ph, pqs, pjp, pcps, pex = pending[0]
                                        emit_c(ph, pjp, pcps, pex)
                                        if pjp == 7:
                                            normalize(ph, pqs, pcps)
                                        pending[0] = None

                                # at jp==0 the pending block is (h-1, jp7): its
                                # normalize must precede any filler D matmul that
                                # reads the outnT rows it writes
                                if jp == 0:
                                    consume()
                                next(filler, None)
                                consume()
                                pending[0] = (h, qs, jp, cps, ex)
                        for _ in filler:
                            pass
                    ph, pqs, pjp, pcps, pex = pending[0]
                    emit_c(ph, pjp, pcps, pex)
                    normalize(ph, pqs, pcps)
                    for _ in d_filler(3, tail=True):
                        pass

    nc.compile()
    return nc


def _get_nc():
    global _NC
    if _NC is None:
        _NC = _build()
    return _NC


def run(inputs, trace=False, trace_cores=None):
    from concourse.bass_utils import run_bass_kernel_spmd

    q = np.asarray(inputs["q"], np.float32)
    k = np.asarray(inputs["k"], np.float32)
    v = np.asarray(inputs["v"], np.float32)
    w_q = np.asarray(inputs["w_q"], np.float32)
    w_k = np.asarray(inputs["w_k"], np.float32)
    w_v = np.asarray(inputs["w_v"], np.float32)
    w_out = np.asarray(inputs["w_out"], np.float32)
    b_q = np.asarray(inputs["b_q"], np.float32)
    b_k = np.asarray(inputs["b_k"], np.float32)
    b_v = np.asarray(inputs["b_v"], np.float32)
    b_out = np.asarray(inputs["b_out"], np.float32)

    import ml_dtypes
    bf16 = ml_dtypes.bfloat16

    xT = {b: {} for b in range(B)}
    for b in range(B):
        xT[b]["qT"] = np.ascontiguousarray(q[b].T.astype(bf16))
        xT[b]["kT"] = np.ascontiguousarray(k[b].T.astype(bf16))
        xT[b]["vT"] = np.ascontiguousarray(v[b].T.astype(bf16))

    in_maps = []
    for c in range(N_CORES):
        b, hq = c // 4, c % 4
        rows = slice(hq * 256, (hq + 1) * 256)
        in_maps.append({
            "qT": xT[b]["qT"],
            "kT": xT[b]["kT"],
            "vT": xT[b]["vT"],
            "wq": np.ascontiguousarray(w_q[rows, :].T.astype(bf16)),
            "wk": np.ascontiguousarray(w_k[rows, :].T.astype(bf16)),
            "wv": np.ascontiguousarray(w_v[rows, :].T.astype(bf16)),
            "wo": np.ascontiguousarray(w_out[:, rows].T.astype(bf16)),
            "bq": np.ascontiguousarray(b_q[rows].reshape(256, 1)),
            "bk": np.ascontiguousarray(b_k[rows].reshape(256, 1)),
            "bv": np.ascontiguousarray(b_v[rows].reshape(1, 256)),
        })

    nc = _get_nc()
    res = run_bass_kernel_spmd(
        nc, in_maps, core_ids=list(range(N_CORES)), trace=trace,
        trace_cores=trace_cores,
    )
    full = np.zeros((B, S, D), np.float32)
    for c in range(N_CORES):
        full[c // 4] += np.asarray(res.results[c]["out"])
    full += b_out.reshape(1, 1, D)
    return full, res.exec_time_ns


def kernel(**inputs):
    return run(inputs, trace=False)[0]
